# revision 1
# baseline (speedup 1.0000x reference)
"""Trainium2 Bass kernel for nn_BlockAttentionResidual.

Reference semantics (per (b, t) position):
    inv_rms_n = rsqrt(mean_d(x_n^2) + eps)                 n = 0..7 sources
    score_n   = dot(q, x_n) * inv_rms_n / sqrt(D)          q = w_query * norm_weight
    w         = softmax_n(score_n)
    out       = sum_n w_n * x_n                            [D]

Sharding: 8192 (b,t) tokens split contiguously across 8 cores (1024 each).
Per core, tokens are processed in 8 "super-iterations" of 128 tokens; each
super-iteration is 8 SBUF tiles of [128 rows = 16 tokens x 8 sources, D=2048].

Tiles stream through a pipeline (load -> fused reductions -> scores -> PE
matmuls) in score-batches of batch_q=2 tiles, so each tile's SBUF slot frees
shortly after its own matmuls retire (keeps the in-order sync-queue DMA
prefetch flowing) while the tiny [128, Q] score ops amortize ScalarE's
per-instruction overhead.  Measured at the pure-DMA roofline for the 72 MiB
of per-core traffic (~220-240 us/core depending on terminal load, ~330 GB/s).

Per-row reductions over D (sum x^2 and dot(q, x)) are single-pass fused ops:
  - ScalarE activation(Square, accum_out=...)        -> sumsq
  - VectorE scalar_tensor_tensor(mult, mult, accum)  -> dot
Softmax skips max-subtraction: |score| <= |q| ~ 0.9 (Cauchy-Schwarz), so exp
is safe.  1/sqrt is computed as exp(-0.5*ln(v)) to stay in one ACT table set.
The weighted combine runs on the PE as 8 PSUM-accumulated matmuls W_j.T @ X_j
with W_j a [128, 128] block-diagonal scatter of exp(score) (built by one
tensor_scalar_mul against a constant mask), in float32r (full-rate fp32
matmul).  The softmax denominator Z accumulates from W_j.T @ ones, and the
PSUM->SBUF eviction applies the 1/Z normalization via a per-partition
activation scale; the store issues from the scalar-engine HWDGE queue so its
wait never stalls the sync queue's load triggers.
"""

import numpy as np

import concourse.bass as bass
import concourse.tile as tile
from concourse import mybir
from concourse.bass_utils import run_bass_kernel_spmd

# Extra kwargs for run_bass_kernel_spmd (test harness sets {"trace": True});
# the last BassKernelResults is stashed for timing inspection.
_run_kwargs = {}
_last_results = None

B, T, N, D = 2, 4096, 8, 2048
EPS = 1e-6
NCORES = 8
TOK = (B * T) // NCORES          # tokens per core = 1024
SUPER = 128                      # tokens per super-iteration
G = TOK // SUPER                 # super-iterations per core = 8
TPT = 128 // N                   # tokens per tile = 16
J = SUPER // TPT                 # tiles per super-iteration = 8

F32 = mybir.dt.float32
F32R = mybir.dt.float32r
FT = mybir.ActivationFunctionType
OP = mybir.AluOpType



def _split_multi_waits(nc: bass.Bass, limit: int = 1) -> None:
    """Move surplus sync waits onto same-engine NoOp carriers.

    This walrus build accepts only one sync-wait slot per ISA instruction;
    Tile can attach several.  A NoOp on the same engine executed immediately
    before the instruction enforces the same AND-of-waits semantics.
    """
    k = 0
    for func in nc.m.functions:
        for blk in func.blocks:
            new_insts = []
            for inst in blk.instructions:
                si = inst.sync_info
                ow = list(si.on_wait) if si is not None and si.on_wait else []
                if len(ow) > limit:
                    for w in ow[:-limit]:
                        nop = mybir.InstNoOp(
                            name=f"waitnop-{k}",
                            sync_info=mybir.SyncInfo(on_wait=[w], on_update=[]),
                            bass_nofuse=True,
                            engine=inst.engine,
                        )
                        k += 1
                        new_insts.append(nop)
                    si.on_wait = ow[-limit:]
                new_insts.append(inst)
            if len(new_insts) != len(blk.instructions):
                blk.instructions[:] = new_insts


def build_nc(split_waits: bool = True, loop_n: int | None = None, batch_q: int = 2, store_scalar: bool = True, body_reps: int = 1) -> bass.Bass:
    nc = bass.Bass()
    src = nc.declare_dram_parameter("src", [TOK * N, D], F32, isOutput=False)
    qv = nc.declare_dram_parameter("qv", [D], F32, isOutput=False)
    maskp = nc.declare_dram_parameter("maskp", [128, J * 128], F32, isOutput=False)
    onesp = nc.declare_dram_parameter("onesp", [128, 2], F32, isOutput=False)
    out = nc.declare_dram_parameter("out", [TOK, D], F32, isOutput=True)

    src_t = src.rearrange("(g j p) d -> g j p d", g=G, j=J, p=128)
    out_t = out.rearrange("(g p) d -> g p d", p=128)

    with tile.TileContext(nc) as tc:
        with (
            tc.tile_pool(name="singles", bufs=1) as singles,
            tc.tile_pool(name="xpool", bufs=18) as xpool,
            tc.tile_pool(name="scratch_a", bufs=1) as scr_a,
            tc.tile_pool(name="scratch_v", bufs=1) as scr_v,
            tc.tile_pool(name="spool", bufs=2) as spool,
            tc.tile_pool(name="wpool", bufs=4) as wpool,
            tc.tile_pool(name="opool", bufs=2) as opool,
            tc.tile_pool(name="psum_o", bufs=1, space="PSUM") as psum_o_pool,
            tc.tile_pool(name="psum_z", bufs=2, space="PSUM") as psum_z_pool,
        ):
            # ---- one-time constants ----
            qb = singles.tile([128, D], F32)
            nc.sync.dma_start(out=qb, in_=qv[None, :].to_broadcast([128, D]))

            mask = singles.tile([128, J * 128], F32)
            nc.sync.dma_start(out=mask, in_=maskp[:, :])

            ones_col = singles.tile([128, 2], F32R)
            nc.sync.dma_start(out=ones_col, in_=onesp[:, :].bitcast(F32R))

            bias_eps = singles.tile([128, 1], F32)
            nc.vector.memset(bias_eps, EPS * D)
            bias_zero = singles.tile([128, 1], F32)
            nc.vector.memset(bias_zero, 0.0)

            # Touch qb on VectorE once so later DVE consumers inherit the
            # dependency via engine program order instead of extra sem waits
            # (the TensorScalarPtr ISA slot has a tight wait budget).
            probe = singles.tile([128, 1], F32)
            nc.vector.tensor_copy(probe, qb[:, 0:1])

            import contextlib

            loop_cm = (
                tc.For_i(0, loop_n, 1,
                         hint_engines=(mybir.EngineType.PE,
                                       mybir.EngineType.Activation,
                                       mybir.EngineType.DVE))
                if loop_n is not None
                else contextlib.nullcontext()
            )
            with loop_cm:
             for _rep in range(body_reps):
              for g in range(G):
                # Per-tile streaming: each tile is loaded, reduced, scored,
                # and fed to the PE immediately, so its SBUF slot frees as
                # soon as its own matmuls retire (keeps DMA prefetch flowing).
                po = psum_o_pool.tile([128, D], F32)
                pz = psum_z_pool.tile([128, 2], F32)
                Q = batch_q  # tiles per score-batch group
                for q0 in range(0, J, Q):
                    xts = []
                    sums = spool.tile([128, Q], F32, tag="sums")
                    dots = spool.tile([128, Q], F32, tag="dots")
                    for k in range(Q):
                        j = q0 + k
                        xt = xpool.tile([128, D], F32R)
                        nc.sync.dma_start(out=xt, in_=src_t[g, j].bitcast(F32R))
                        xts.append(xt)
                        sq_scr = scr_a.tile([128, D], F32, tag="sq")
                        nc.scalar.activation(
                            out=sq_scr,
                            in_=xt.bitcast(F32),
                            func=FT.Square,
                            accum_out=sums[:, k : k + 1],
                        )
                        tt_scr = scr_v.tile([128, D], F32, tag="tt")
                        nc.vector.scalar_tensor_tensor(
                            out=tt_scr,
                            in0=xt.bitcast(F32),
                            scalar=1.0,
                            in1=qb,
                            op0=OP.mult,
                            op1=OP.mult,
                            accum_out=dots[:, k : k + 1],
                        )

                    # score = dot / sqrt(sumsq + eps*D); 1/sqrt = exp(-0.5*ln)
                    lnv = spool.tile([128, Q], F32, tag="lnv")
                    nc.scalar.activation(
                        out=lnv, in_=sums, func=FT.Ln, bias=bias_eps, scale=1.0
                    )
                    rhat = spool.tile([128, Q], F32, tag="rhat")
                    nc.scalar.activation(
                        out=rhat, in_=lnv, func=FT.Exp, bias=bias_zero, scale=-0.5
                    )
                    scores = spool.tile([128, Q], F32, tag="scores")
                    nc.vector.tensor_mul(scores, dots, rhat)
                    evals = spool.tile([128, Q], F32, tag="evals")
                    nc.scalar.activation(
                        out=evals, in_=scores, func=FT.Exp, bias=bias_zero
                    )

                    for k in range(Q):
                        j = q0 + k
                        w = wpool.tile([128, 128], F32R, tag="w")
                        nc.vector.tensor_scalar_mul(
                            w, mask[:, 128 * j : 128 * (j + 1)],
                            evals[:, k : k + 1],
                        )
                        for c in range(D // 512):
                            nc.tensor.matmul(
                                po[:, 512 * c : 512 * (c + 1)],
                                w,
                                xts[k][:, 512 * c : 512 * (c + 1)],
                                start=(j == 0),
                                stop=(j == J - 1),
                            )
                        nc.tensor.matmul(
                            pz, w, ones_col, start=(j == 0), stop=(j == J - 1)
                        )

                # ---- normalize by Z during PSUM eviction, then store ----
                invz = spool.tile([128, 1], F32, tag="invz")
                nc.vector.reciprocal(invz, pz[:, 0:1])
                ot = opool.tile([128, D], F32)
                nc.scalar.activation(out=ot, in_=po, func=FT.Copy, scale=invz)
                # Store via the scalar-engine HWDGE queue: its wait (evict
                # done) is satisfied by ACT program order, so it never blocks
                # the sync queue's load triggers for the next super-iter.
                store_eng = nc.scalar if store_scalar else nc.sync
                store_eng.dma_start(out=out_t[g], in_=ot)

    if split_waits:
        _split_multi_waits(nc)
    return nc


def make_mask() -> np.ndarray:
    """Block-diagonal weight scatter masks, one [128, 128] block per tile j.

    Block j has mask[p, TPT*j + p // N] = 1: row p of tile j (= token p//N,
    source p%N) contributes to output token TPT*j + p//N of the super-iter.
    """
    m = np.zeros((128, J * 128), dtype=np.float32)
    for j in range(J):
        for p in range(128):
            m[p, 128 * j + TPT * j + p // N] = 1.0
    return m


def kernel(sources, w_query, norm_weight):
    sources = np.asarray(sources, dtype=np.float32)
    w_query = np.asarray(w_query, dtype=np.float32)
    norm_weight = np.asarray(norm_weight, dtype=np.float32)

    nc = build_nc()

    q = np.ascontiguousarray(w_query * norm_weight)
    flat = np.ascontiguousarray(sources.reshape(B * T * N, D))
    mask_np = make_mask()
    ones_np = np.ones((128, 2), dtype=np.float32)
    in_maps = [
        {"src": flat[c * TOK * N : (c + 1) * TOK * N], "qv": q, "maskp": mask_np,
         "onesp": ones_np}
        for c in range(NCORES)
    ]
    global _last_results
    res = run_bass_kernel_spmd(nc, in_maps, list(range(NCORES)), **_run_kwargs)
    _last_results = res
    outs = [res.results[c]["out"] for c in range(NCORES)]
    return np.concatenate(outs, axis=0).reshape(B, T, D).astype(np.float32)



# revision 12
# speedup vs baseline: 1.0638x; 1.0638x over previous
"""Trainium2 Bass kernel for nn_BlockAttentionResidual.

Reference semantics (per (b, t) position):
    inv_rms_n = rsqrt(mean_d(x_n^2) + eps)                 n = 0..7 sources
    score_n   = dot(q, x_n) * inv_rms_n / sqrt(D)          q = w_query * norm_weight
    w         = softmax_n(score_n)
    out       = sum_n w_n * x_n                            [D]

Sharding: 8192 (b,t) tokens split contiguously across 8 cores (1024 each).

The kernel streams fp16 inputs (converted on the host inside kernel(); the
tolerance budget easily covers fp16 rounding, ~5e-4 end-to-end rel err) which
halves the dominant HBM read traffic vs fp32: 32 MiB in + 4 MiB out per core.

Per core, tokens are processed in 8 super-iterations of 128 tokens; each
super-iteration is J=8 SBUF tiles of [128 rows = 16 tokens x 8 sources, D].
Per tile, two full-width reduction passes are needed (sum x^2 and dot(q, x));
at the fp16 DMA roofline (~110-120 us) a single engine cannot cover the
2*64 = 128 tile-passes per core, so they are split across ScalarE
(activation Square + accum), VectorE (scalar_tensor_tensor + accum) and
GpSimd (same STT op, slower software rate) according to a static schedule
(counts tuned on hardware).

Softmax skips max-subtraction: |score| <= |q| ~ 0.9.  1/sqrt is computed as
exp(-0.5*ln(v)).  The weighted combine runs on the PE as PSUM-accumulated
matmuls W_j.T @ X_j in fp16 (full rate, 1 col/cycle), with W_j a [128, 128]
block-diagonal scatter of exp(score) built by one tensor_scalar_mul against
a constant mask.  The softmax denominator Z accumulates from W_j.T @ ones;
the PSUM->SBUF eviction applies 1/Z via a per-partition activation scale and
emits fp16, stored from the scalar-engine HWDGE queue.
"""

import numpy as np

import concourse.bass as bass
import concourse.tile as tile
from concourse import mybir
from concourse.bass_utils import run_bass_kernel_spmd

# Extra kwargs for run_bass_kernel_spmd (test harness sets {"trace": True});
# the last BassKernelResults is stashed for timing inspection.
_run_kwargs = {}
_last_results = None

B, T, N, D = 2, 4096, 8, 2048
EPS = 1e-6
NCORES = 8
TOK = (B * T) // NCORES          # tokens per core = 1024
SUPER = 128                      # tokens per super-iteration
G = TOK // SUPER                 # super-iterations per core = 8
TPT = 128 // N                   # tokens per tile = 16
J = SUPER // TPT                 # tiles per super-iteration = 8
NT = G * J                       # tiles per core = 64

F32 = mybir.dt.float32
FP16 = mybir.dt.float16
FT = mybir.ActivationFunctionType
OP = mybir.AluOpType

# Reduction-pass schedule: which engine does each tile's sumsq / dot.
# 'A' = ScalarE activation(Square), 'V' = VectorE STT, 'P' = GpSimd STT.
SUMSQ_SPLIT = {"A": 58, "V": 6, "P": 0}    # must sum to NT
DOT_SPLIT = {"V": 64, "P": 0}              # must sum to NT


def _spread(split: dict[str, int], n: int) -> list[str]:
    """Interleave engine assignments evenly across n slots."""
    assert sum(split.values()) == n
    acc = {k: 0.0 for k in split}
    out = []
    for _ in range(n):
        for k in acc:
            acc[k] += split[k] / n
        k = max(acc, key=lambda e: acc[e])
        out.append(k)
        acc[k] -= 1.0
    counts = {k: out.count(k) for k in split}
    assert counts == split, (counts, split)
    return out


def _make_schedule(sumsq_split=None, dot_split=None):
    ss = _spread(sumsq_split or SUMSQ_SPLIT, NT)
    dd = _spread(dot_split or DOT_SPLIT, NT)
    return ss, dd


def _split_multi_waits(nc: bass.Bass, limit: int = 1) -> None:
    """Move surplus sync waits onto same-engine NoOp carriers.

    This walrus build accepts only one sync-wait slot per ISA instruction;
    Tile can attach several.  A NoOp on the same engine executed immediately
    before the instruction enforces the same AND-of-waits semantics.
    """
    k = 0
    for func in nc.m.functions:
        for blk in func.blocks:
            new_insts = []
            for inst in blk.instructions:
                si = inst.sync_info
                ow = list(si.on_wait) if si is not None and si.on_wait else []
                if len(ow) > limit:
                    for w in ow[:-limit]:
                        nop = mybir.InstNoOp(
                            name=f"waitnop-{k}",
                            sync_info=mybir.SyncInfo(on_wait=[w], on_update=[]),
                            bass_nofuse=True,
                            engine=inst.engine,
                        )
                        k += 1
                        new_insts.append(nop)
                    si.on_wait = ow[-limit:]
                new_insts.append(inst)
            if len(new_insts) != len(blk.instructions):
                blk.instructions[:] = new_insts


def build_nc(split_waits: bool = True, loop_n: int | None = None,
             store_scalar: bool = True, body_reps: int = 1,
             sumsq_split=None, dot_split=None, xbufs: int = 22) -> bass.Bass:
    ss_eng, dot_eng = _make_schedule(sumsq_split, dot_split)

    nc = bass.Bass()
    src = nc.declare_dram_parameter("src", [TOK * N, D], FP16, isOutput=False)
    qv = nc.declare_dram_parameter("qv", [D], FP16, isOutput=False)
    maskp = nc.declare_dram_parameter("maskp", [128, J * 128], FP16, isOutput=False)
    onesp = nc.declare_dram_parameter("onesp", [128, 2], FP16, isOutput=False)
    out = nc.declare_dram_parameter("out", [TOK, D], FP16, isOutput=True)

    src_t = src.rearrange("(g j p) d -> g j p d", g=G, j=J, p=128)
    out_t = out.rearrange("(g p) d -> g p d", p=128)

    with tile.TileContext(nc) as tc:
        with (
            tc.tile_pool(name="singles", bufs=1) as singles,
            tc.tile_pool(name="xpool", bufs=xbufs) as xpool,
            tc.tile_pool(name="scr_a", bufs=1) as scr_a,
            tc.tile_pool(name="scr_v", bufs=1) as scr_v,
            tc.tile_pool(name="scr_p", bufs=1) as scr_p,
            tc.tile_pool(name="spool", bufs=2) as spool,
            tc.tile_pool(name="wpool", bufs=4) as wpool,
            tc.tile_pool(name="opool", bufs=2) as opool,
            tc.tile_pool(name="psum_o", bufs=1, space="PSUM") as psum_o_pool,
            tc.tile_pool(name="psum_z", bufs=2, space="PSUM") as psum_z_pool,
        ):
            # ---- one-time constants ----
            qb = singles.tile([128, D], FP16)
            nc.sync.dma_start(out=qb, in_=qv[None, :].to_broadcast([128, D]))

            mask = singles.tile([128, J * 128], FP16)
            nc.sync.dma_start(out=mask, in_=maskp[:, :])

            ones_col = singles.tile([128, 2], FP16)
            nc.sync.dma_start(out=ones_col, in_=onesp[:, :])

            bias_eps = singles.tile([128, 1], F32)
            nc.vector.memset(bias_eps, EPS * D)
            bias_zero = singles.tile([128, 1], F32)
            nc.vector.memset(bias_zero, 0.0)

            # Touch qb on VectorE once so later consumers inherit the
            # dependency via engine program order instead of extra sem waits
            # (the TensorScalarPtr ISA slot has a tight wait budget).
            probe = singles.tile([128, 2], F32)
            nc.vector.tensor_copy(probe[:, 0:1], qb[:, 0:1])
            if "P" in ss_eng or "P" in dot_eng:
                nc.gpsimd.tensor_copy(probe[:, 1:2], qb[:, 0:1])

            import contextlib

            loop_cm = (
                tc.For_i(0, loop_n, 1,
                         hint_engines=(mybir.EngineType.PE,
                                       mybir.EngineType.Activation,
                                       mybir.EngineType.DVE,
                                       mybir.EngineType.Pool))
                if loop_n is not None
                else contextlib.nullcontext()
            )
            # ---- per-super emission stages (2-deep software pipeline) ----

            def emit_loads_reductions(g):
                sums = spool.tile([128, J], F32, tag="sums")
                dots = spool.tile([128, J], F32, tag="dots")
                xts = []
                for j in range(J):
                    i = g * J + j
                    xt = xpool.tile([128, D], FP16)
                    nc.sync.dma_start(out=xt, in_=src_t[g, j])
                    xts.append(xt)

                    se = ss_eng[i]
                    if se == "A":
                        sq_scr = scr_a.tile([128, D], FP16, tag="sq")
                        nc.scalar.activation(
                            out=sq_scr, in_=xt, func=FT.Square,
                            bias=bias_zero, scale=1.0,
                            accum_out=sums[:, j : j + 1],
                        )
                    else:
                        eng = nc.vector if se == "V" else nc.gpsimd
                        scr = (scr_v if se == "V" else scr_p).tile(
                            [128, D], FP16, tag="sq")
                        eng.scalar_tensor_tensor(
                            out=scr, in0=xt, scalar=1.0, in1=xt,
                            op0=OP.mult, op1=OP.mult,
                            accum_out=sums[:, j : j + 1],
                        )

                    de = dot_eng[i]
                    eng = nc.vector if de == "V" else nc.gpsimd
                    scr = (scr_v if de == "V" else scr_p).tile(
                        [128, D], FP16, tag="tt")
                    eng.scalar_tensor_tensor(
                        out=scr, in0=xt, scalar=1.0, in1=qb,
                        op0=OP.mult, op1=OP.mult,
                        accum_out=dots[:, j : j + 1],
                    )
                return sums, dots, xts

            def emit_scores(st):
                # score = dot / sqrt(sumsq + eps*D); 1/sqrt = exp(-0.5*ln)
                sums, dots = st["sums"], st["dots"]
                lnv = spool.tile([128, J], F32, tag="lnv")
                nc.scalar.activation(
                    out=lnv, in_=sums, func=FT.Ln, bias=bias_eps, scale=1.0
                )
                rhat = spool.tile([128, J], F32, tag="rhat")
                nc.scalar.activation(
                    out=rhat, in_=lnv, func=FT.Exp, bias=bias_zero, scale=-0.5
                )
                scores = spool.tile([128, J], F32, tag="scores")
                nc.vector.tensor_mul(scores, dots, rhat)
                evals = spool.tile([128, J], F32, tag="evals")
                nc.scalar.activation(
                    out=evals, in_=scores, func=FT.Exp, bias=bias_zero
                )
                st["evals"] = evals

            def emit_matmuls(st):
                po = psum_o_pool.tile([128, D], F32)
                pz = psum_z_pool.tile([128, 2], F32)
                evals, xts = st["evals"], st["xts"]
                for j in range(J):
                    w = wpool.tile([128, 128], FP16, tag="w")
                    nc.vector.tensor_scalar_mul(
                        w, mask[:, 128 * j : 128 * (j + 1)],
                        evals[:, j : j + 1],
                    )
                    for c in range(D // 512):
                        nc.tensor.matmul(
                            po[:, 512 * c : 512 * (c + 1)],
                            w,
                            xts[j][:, 512 * c : 512 * (c + 1)],
                            start=(j == 0),
                            stop=(j == J - 1),
                        )
                    nc.tensor.matmul(
                        pz, w, ones_col, start=(j == 0), stop=(j == J - 1)
                    )
                st["po"], st["pz"] = po, pz

            def emit_recip(st):
                invz = spool.tile([128, 1], F32, tag="invz")
                nc.vector.reciprocal(invz, st["pz"][:, 0:1])
                st["invz"] = invz

            def emit_evict(st):
                ot = opool.tile([128, D], FP16)
                nc.scalar.activation(
                    out=ot, in_=st["po"], func=FT.Copy, scale=st["invz"])
                # Store via the scalar-engine HWDGE queue: its wait (evict
                # done) is satisfied by ACT program order, so it never blocks
                # the sync queue's load triggers.
                store_eng = nc.scalar if store_scalar else nc.sync
                store_eng.dma_start(out=out_t[st["g"]], in_=ot)

            with loop_cm:
             for _rep in range(body_reps):
              prev = None   # super g-1: loaded+reduced, needs scores+matmuls
              done = None   # super g-2: matmuls queued, needs recip+evict
              for g in range(G):
                sums, dots, xts = emit_loads_reductions(g)
                cur = {"g": g, "sums": sums, "dots": dots, "xts": xts}
                if prev is not None:
                    emit_scores(prev)
                if done is not None:
                    # recip on DVE before ACT needs it for the eviction; the
                    # PSUM source was finished a full super ago, so neither
                    # engine blocks here.
                    emit_recip(done)
                if prev is not None:
                    if done is not None:
                        emit_evict(done)
                    emit_matmuls(prev)
                done, prev = prev, cur
              # drain: scores+matmuls for the last super, evictions for both
              emit_scores(prev)
              emit_recip(done)
              emit_evict(done)
              emit_matmuls(prev)
              emit_recip(prev)
              emit_evict(prev)

    if split_waits:
        _split_multi_waits(nc)
    return nc


def make_mask() -> np.ndarray:
    """Block-diagonal weight scatter masks, one [128, 128] block per tile j.

    Block j has mask[p, TPT*j + p // N] = 1: row p of tile j (= token p//N,
    source p%N) contributes to output token TPT*j + p//N of the super-iter.
    """
    m = np.zeros((128, J * 128), dtype=np.float16)
    for j in range(J):
        for p in range(128):
            m[p, 128 * j + TPT * j + p // N] = 1.0
    return m


def kernel(sources, w_query, norm_weight):
    sources = np.asarray(sources, dtype=np.float32)
    w_query = np.asarray(w_query, dtype=np.float32)
    norm_weight = np.asarray(norm_weight, dtype=np.float32)

    nc = build_nc()

    q = np.ascontiguousarray((w_query * norm_weight).astype(np.float16))
    flat = np.ascontiguousarray(
        sources.reshape(B * T * N, D).astype(np.float16))
    mask_np = make_mask()
    ones_np = np.ones((128, 2), dtype=np.float16)
    in_maps = [
        {"src": flat[c * TOK * N : (c + 1) * TOK * N], "qv": q,
         "maskp": mask_np, "onesp": ones_np}
        for c in range(NCORES)
    ]
    global _last_results
    res = run_bass_kernel_spmd(nc, in_maps, list(range(NCORES)), **_run_kwargs)
    _last_results = res
    outs = [res.results[c]["out"] for c in range(NCORES)]
    return (
        np.concatenate(outs, axis=0).reshape(B, T, D).astype(np.float32)
    )


# revision 15
# speedup vs baseline: 1.2656x; 1.1898x over previous
"""Trainium2 Bass kernel for nn_BlockAttentionResidual.

Reference semantics (per (b, t) position):
    inv_rms_n = rsqrt(mean_d(x_n^2) + eps)                 n = 0..7 sources
    score_n   = dot(q, x_n) * inv_rms_n / sqrt(D)          q = w_query * norm_weight
    w         = softmax_n(score_n)
    out       = sum_n w_n * x_n                            [D]

Sharding: 8192 (b,t) tokens split contiguously across 8 cores (1024 each).

The kernel streams fp16 inputs (converted on the host inside kernel(); the
2e-2 tolerance easily covers fp16 rounding, ~5e-4 end-to-end rel err), which
halves the dominant HBM read traffic vs fp32: 32 MiB in + 4 MiB out per core
(~105-120 us of DMA at the ~330 GB/s per-core rate).

Per core, tokens are processed in 8 super-iterations of 128 tokens; each is
J=8 SBUF tiles of [128 rows = 16 tokens x 8 sources, D].  The binding
constraint is the two full-width reduction passes per tile (sum x^2 and
dot(q, x)): on this hardware every reduction-capable op runs at 1 elem/
lane/cycle (DVE scalar_tensor_tensor has no 16-bit packing mode, ScalarE
ACTIVATE is dtype-independent, GpSimd cannot run TensorScalarPtr at all, and
the PE only contracts over partitions so neither reduction can use it), so
the 128 passes are split between ScalarE (activation Square + accum,
~2.36 us) and VectorE (STT + accum, ~2.26 us) by a static schedule tuned on
hardware; the dot can only run on VectorE, which pins DVE at ~150 us and
makes ~165 us the compute floor for this op set.

Emission is software-pipelined two supers deep (reductions for super g,
then scores for g-1, then eviction for g-2, then matmuls for g-1) so the
in-order ACT/DVE instruction streams always have productive work queued
ahead of any cross-engine wait.  Softmax skips max-subtraction:
|score| <= |q| ~ 0.9.  1/sqrt is computed as exp(-0.5*ln(v)) so Square/Ln/
Exp/Copy stay in one ACT table set (no 1.3 us table reloads).  The weighted
combine runs on the PE as PSUM-accumulated matmuls W_j.T @ X_j in fp16
(1 col/cycle, moving operand <= 512 cols for fp16), with W_j a [128, 128]
block-diagonal scatter of exp(score) built by one tensor_scalar_mul against
a constant mask.  The softmax denominator Z accumulates from W_j.T @ ones;
the PSUM->SBUF eviction applies 1/Z via a per-partition activation scale and
emits fp16, stored from the scalar-engine HWDGE queue.
"""

import numpy as np

import concourse.bass as bass
import concourse.tile as tile
from concourse import mybir
from concourse.bass_utils import run_bass_kernel_spmd

# Extra kwargs for run_bass_kernel_spmd (test harness sets {"trace": True});
# the last BassKernelResults is stashed for timing inspection.
_run_kwargs = {}
_last_results = None

B, T, N, D = 2, 4096, 8, 2048
EPS = 1e-6
NCORES = 8
TOK = (B * T) // NCORES          # tokens per core = 1024
SUPER = 128                      # tokens per super-iteration
G = TOK // SUPER                 # super-iterations per core = 8
TPT = 128 // N                   # tokens per tile = 16
J = SUPER // TPT                 # tiles per super-iteration = 8
NT = G * J                       # tiles per core = 64

F32 = mybir.dt.float32
import os
DT16_NAME = os.environ.get("K_DT16", "float16")
FP16 = mybir.dt.float16 if DT16_NAME == "float16" else mybir.dt.bfloat16
NP16 = __import__("numpy").float16 if DT16_NAME == "float16" else __import__("ml_dtypes").bfloat16
FT = mybir.ActivationFunctionType
OP = mybir.AluOpType

# Reduction-pass schedule: which engine does each tile's sumsq / dot.
# 'A' = ScalarE activation(Square), 'V' = VectorE STT, 'P' = GpSimd STT.
SUMSQ_SPLIT = {"A": 61, "V": 3, "P": 0}    # must sum to NT
DOT_SPLIT = {"V": 64, "P": 0}              # must sum to NT


def _spread(split: dict[str, int], n: int) -> list[str]:
    """Interleave engine assignments evenly across n slots."""
    assert sum(split.values()) == n
    acc = {k: 0.0 for k in split}
    out = []
    for _ in range(n):
        for k in acc:
            acc[k] += split[k] / n
        k = max(acc, key=lambda e: acc[e])
        out.append(k)
        acc[k] -= 1.0
    counts = {k: out.count(k) for k in split}
    assert counts == split, (counts, split)
    return out


def _make_schedule(sumsq_split=None, dot_split=None):
    ss = _spread(sumsq_split or SUMSQ_SPLIT, NT)
    dd = _spread(dot_split or DOT_SPLIT, NT)
    return ss, dd


def _split_multi_waits(nc: bass.Bass, limit: int = 1) -> None:
    """Move surplus sync waits onto same-engine NoOp carriers.

    This walrus build accepts only one sync-wait slot per ISA instruction;
    Tile can attach several.  A NoOp on the same engine executed immediately
    before the instruction enforces the same AND-of-waits semantics.
    """
    k = 0
    for func in nc.m.functions:
        for blk in func.blocks:
            new_insts = []
            for inst in blk.instructions:
                si = inst.sync_info
                ow = list(si.on_wait) if si is not None and si.on_wait else []
                if len(ow) > limit:
                    for w in ow[:-limit]:
                        nop = mybir.InstNoOp(
                            name=f"waitnop-{k}",
                            sync_info=mybir.SyncInfo(on_wait=[w], on_update=[]),
                            bass_nofuse=True,
                            engine=inst.engine,
                        )
                        k += 1
                        new_insts.append(nop)
                    si.on_wait = ow[-limit:]
                new_insts.append(inst)
            if len(new_insts) != len(blk.instructions):
                blk.instructions[:] = new_insts


def build_nc(split_waits: bool = True, loop_n: int | None = None,
             store_scalar: bool = True, body_reps: int = 1,
             sumsq_split=None, dot_split=None, xbufs: int = 22,
             spool_bufs: int = 3, wpool_bufs: int = 8,
             opool_bufs: int = 2) -> bass.Bass:
    ss_eng, dot_eng = _make_schedule(sumsq_split, dot_split)

    nc = bass.Bass()
    src = nc.declare_dram_parameter("src", [TOK * N, D], FP16, isOutput=False)
    qv = nc.declare_dram_parameter("qv", [D], FP16, isOutput=False)
    maskp = nc.declare_dram_parameter("maskp", [128, J * 128], FP16, isOutput=False)
    onesp = nc.declare_dram_parameter("onesp", [128, 2], FP16, isOutput=False)
    out = nc.declare_dram_parameter("out", [TOK, D], FP16, isOutput=True)

    src_t = src.rearrange("(g j p) d -> g j p d", g=G, j=J, p=128)
    out_t = out.rearrange("(g p) d -> g p d", p=128)

    with tile.TileContext(nc) as tc:
        with (
            tc.tile_pool(name="singles", bufs=1) as singles,
            tc.tile_pool(name="xpool", bufs=xbufs) as xpool,
            tc.tile_pool(name="scr_a", bufs=1) as scr_a,
            tc.tile_pool(name="scr_v", bufs=1) as scr_v,
            tc.tile_pool(name="scr_p", bufs=1) as scr_p,
            tc.tile_pool(name="spool", bufs=spool_bufs) as spool,
            tc.tile_pool(name="wpool", bufs=wpool_bufs) as wpool,
            tc.tile_pool(name="opool", bufs=opool_bufs) as opool,
            tc.tile_pool(name="psum_o", bufs=1, space="PSUM") as psum_o_pool,
            tc.tile_pool(name="psum_z", bufs=2, space="PSUM") as psum_z_pool,
        ):
            # ---- one-time constants ----
            qb = singles.tile([128, D], FP16)
            nc.sync.dma_start(out=qb, in_=qv[None, :].to_broadcast([128, D]))

            mask = singles.tile([128, J * 128], FP16)
            nc.sync.dma_start(out=mask, in_=maskp[:, :])

            ones_col = singles.tile([128, 2], FP16)
            nc.sync.dma_start(out=ones_col, in_=onesp[:, :])

            bias_eps = singles.tile([128, 1], F32)
            nc.vector.memset(bias_eps, EPS * D)
            bias_zero = singles.tile([128, 1], F32)
            nc.vector.memset(bias_zero, 0.0)

            # Touch qb on VectorE once so later consumers inherit the
            # dependency via engine program order instead of extra sem waits
            # (the TensorScalarPtr ISA slot has a tight wait budget).
            probe = singles.tile([128, 2], F32)
            nc.vector.tensor_copy(probe[:, 0:1], qb[:, 0:1])
            if "P" in ss_eng or "P" in dot_eng:
                nc.gpsimd.tensor_copy(probe[:, 1:2], qb[:, 0:1])

            import contextlib

            loop_cm = (
                tc.For_i(0, loop_n, 1,
                         hint_engines=(mybir.EngineType.PE,
                                       mybir.EngineType.Activation,
                                       mybir.EngineType.DVE,
                                       mybir.EngineType.Pool))
                if loop_n is not None
                else contextlib.nullcontext()
            )
            # ---- per-super emission stages (2-deep software pipeline) ----

            def emit_loads_reductions(g):
                sums = spool.tile([128, J], F32, tag="sums")
                dots = spool.tile([128, J], F32, tag="dots")
                xts = []
                for j in range(J):
                    i = g * J + j
                    xt = xpool.tile([128, D], FP16)
                    nc.sync.dma_start(out=xt, in_=src_t[g, j])
                    xts.append(xt)

                    se = ss_eng[i]
                    if se == "A":
                        sq_scr = scr_a.tile([128, D], FP16, tag="sq")
                        nc.scalar.activation(
                            out=sq_scr, in_=xt, func=FT.Square,
                            bias=bias_zero, scale=1.0,
                            accum_out=sums[:, j : j + 1],
                        )
                    else:
                        eng = nc.vector if se == "V" else nc.gpsimd
                        scr = (scr_v if se == "V" else scr_p).tile(
                            [128, D], FP16, tag="sq")
                        eng.scalar_tensor_tensor(
                            out=scr, in0=xt, scalar=1.0, in1=xt,
                            op0=OP.mult, op1=OP.mult,
                            accum_out=sums[:, j : j + 1],
                        )

                    de = dot_eng[i]
                    eng = nc.vector if de == "V" else nc.gpsimd
                    scr = (scr_v if de == "V" else scr_p).tile(
                        [128, D], FP16, tag="tt")
                    eng.scalar_tensor_tensor(
                        out=scr, in0=xt, scalar=1.0, in1=qb,
                        op0=OP.mult, op1=OP.mult,
                        accum_out=dots[:, j : j + 1],
                    )
                return sums, dots, xts

            def emit_scores(st):
                # score = dot / sqrt(sumsq + eps*D); 1/sqrt = exp(-0.5*ln)
                sums, dots = st["sums"], st["dots"]
                lnv = spool.tile([128, J], F32, tag="lnv")
                nc.scalar.activation(
                    out=lnv, in_=sums, func=FT.Ln, bias=bias_eps, scale=1.0
                )
                rhat = spool.tile([128, J], F32, tag="rhat")
                nc.scalar.activation(
                    out=rhat, in_=lnv, func=FT.Exp, bias=bias_zero, scale=-0.5
                )
                scores = spool.tile([128, J], F32, tag="scores")
                nc.vector.tensor_mul(scores, dots, rhat)
                evals = spool.tile([128, J], F32, tag="evals")
                nc.scalar.activation(
                    out=evals, in_=scores, func=FT.Exp, bias=bias_zero
                )
                st["evals"] = evals

            def emit_matmuls(st):
                po = psum_o_pool.tile([128, D], F32)
                pz = psum_z_pool.tile([128, 2], F32)
                evals, xts = st["evals"], st["xts"]
                for j in range(J):
                    w = wpool.tile([128, 128], FP16, tag="w")
                    nc.vector.tensor_scalar_mul(
                        w, mask[:, 128 * j : 128 * (j + 1)],
                        evals[:, j : j + 1],
                    )
                    for c in range(D // 512):
                        nc.tensor.matmul(
                            po[:, 512 * c : 512 * (c + 1)],
                            w,
                            xts[j][:, 512 * c : 512 * (c + 1)],
                            start=(j == 0),
                            stop=(j == J - 1),
                        )
                    nc.tensor.matmul(
                        pz, w, ones_col, start=(j == 0), stop=(j == J - 1)
                    )
                st["po"], st["pz"] = po, pz

            def emit_recip(st):
                invz = spool.tile([128, 1], F32, tag="invz")
                nc.vector.reciprocal(invz, st["pz"][:, 0:1])
                st["invz"] = invz

            def emit_evict(st):
                ot = opool.tile([128, D], FP16)
                nc.scalar.activation(
                    out=ot, in_=st["po"], func=FT.Copy, scale=st["invz"])
                # Store via the scalar-engine HWDGE queue: its wait (evict
                # done) is satisfied by ACT program order, so it never blocks
                # the sync queue's load triggers.
                store_eng = nc.scalar if store_scalar else nc.sync
                store_eng.dma_start(out=out_t[st["g"]], in_=ot)

            with loop_cm:
             for _rep in range(body_reps):
              prev = None   # super g-1: loaded+reduced, needs scores+matmuls
              done = None   # super g-2: matmuls queued, needs recip+evict
              for g in range(G):
                sums, dots, xts = emit_loads_reductions(g)
                cur = {"g": g, "sums": sums, "dots": dots, "xts": xts}
                if prev is not None:
                    emit_scores(prev)
                if done is not None:
                    # recip on DVE before ACT needs it for the eviction; the
                    # PSUM source was finished a full super ago, so neither
                    # engine blocks here.
                    emit_recip(done)
                if prev is not None:
                    if done is not None:
                        emit_evict(done)
                    emit_matmuls(prev)
                done, prev = prev, cur
              # drain: scores+matmuls for the last super, evictions for both
              emit_scores(prev)
              emit_recip(done)
              emit_evict(done)
              emit_matmuls(prev)
              emit_recip(prev)
              emit_evict(prev)

    if split_waits:
        _split_multi_waits(nc)
    return nc


def make_mask() -> np.ndarray:
    """Block-diagonal weight scatter masks, one [128, 128] block per tile j.

    Block j has mask[p, TPT*j + p // N] = 1: row p of tile j (= token p//N,
    source p%N) contributes to output token TPT*j + p//N of the super-iter.
    """
    m = np.zeros((128, J * 128), dtype=NP16)
    for j in range(J):
        for p in range(128):
            m[p, 128 * j + TPT * j + p // N] = 1.0
    return m


def kernel(sources, w_query, norm_weight):
    sources = np.asarray(sources, dtype=np.float32)
    w_query = np.asarray(w_query, dtype=np.float32)
    norm_weight = np.asarray(norm_weight, dtype=np.float32)

    nc = build_nc()

    q = np.ascontiguousarray((w_query * norm_weight).astype(NP16))
    flat = np.ascontiguousarray(
        sources.reshape(B * T * N, D).astype(NP16))
    mask_np = make_mask()
    ones_np = np.ones((128, 2), dtype=NP16)
    in_maps = [
        {"src": flat[c * TOK * N : (c + 1) * TOK * N], "qv": q,
         "maskp": mask_np, "onesp": ones_np}
        for c in range(NCORES)
    ]
    global _last_results
    res = run_bass_kernel_spmd(nc, in_maps, list(range(NCORES)), **_run_kwargs)
    _last_results = res
    outs = [res.results[c]["out"] for c in range(NCORES)]
    return (
        np.concatenate(outs, axis=0).reshape(B, T, D).astype(np.float32)
    )


# revision 20
# speedup vs baseline: 1.3036x; 1.0300x over previous
"""Trainium2 Bass kernel for nn_BlockAttentionResidual.

Reference semantics (per (b, t) position):
    inv_rms_n = rsqrt(mean_d(x_n^2) + eps)                 n = 0..7 sources
    score_n   = dot(q, x_n) * inv_rms_n / sqrt(D)          q = w_query * norm_weight
    w         = softmax_n(score_n)
    out       = sum_n w_n * x_n                            [D]

Sharding: 8192 (b,t) tokens split contiguously across 8 cores (1024 each).

The kernel streams fp16 inputs (converted on the host inside kernel(); the
2e-2 tolerance easily covers fp16 rounding, ~5e-4 end-to-end rel err), which
halves the dominant HBM read traffic vs fp32: 32 MiB in + 4 MiB out per core
(~105-120 us of DMA at the ~330 GB/s per-core rate).

Per core, tokens are processed in 8 super-iterations of 128 tokens; each is
J=8 SBUF tiles of [128 rows = 16 tokens x 8 sources, D].  The binding
constraint is the two full-width reduction passes per tile (sum x^2 and
dot(q, x)): on this hardware every reduction-capable op runs at 1 elem/
lane/cycle (DVE scalar_tensor_tensor has no 16-bit packing mode, ScalarE
ACTIVATE is dtype-independent, GpSimd cannot run TensorScalarPtr at all, and
the PE only contracts over partitions so neither reduction can use it), so
the 128 passes are split between ScalarE (activation Square + accum,
~2.36 us) and VectorE (STT + accum, ~2.26 us) by a static schedule tuned on
hardware; the dot can only run on VectorE, which pins DVE at ~150 us and
makes ~165 us the compute floor for this op set.  GpSimd full-width
tensor_tensor measured ~4x slower than its cost-model rate, so it only
carries the tiny per-super scores multiply (dots * rhat, [128, 8]), which
removes a serialization point from the VectorE stream (~8 us).

Emission is software-pipelined two supers deep (reductions for super g,
then scores for g-1, then eviction for g-2, then matmuls for g-1) so the
in-order ACT/DVE instruction streams always have productive work queued
ahead of any cross-engine wait.  Softmax skips max-subtraction:
|score| <= |q| ~ 0.9.  1/sqrt is computed as exp(-0.5*ln(v)) so Square/Ln/
Exp/Copy stay in one ACT table set (no 1.3 us table reloads).  The weighted
combine runs on the PE as PSUM-accumulated matmuls W_j.T @ X_j in fp16
(1 col/cycle, moving operand <= 512 cols for fp16), with W_j a [128, 128]
block-diagonal scatter of exp(score) built by one tensor_scalar_mul against
a constant mask.  The softmax denominator Z accumulates from W_j.T @ ones;
the PSUM->SBUF eviction applies 1/Z via a per-partition activation scale and
emits fp16, stored from the scalar-engine HWDGE queue.
"""

import numpy as np

import concourse.bass as bass
import concourse.tile as tile
from concourse import mybir
from concourse.bass_utils import run_bass_kernel_spmd

# Extra kwargs for run_bass_kernel_spmd (test harness sets {"trace": True});
# the last BassKernelResults is stashed for timing inspection.
_run_kwargs = {}
_last_results = None

B, T, N, D = 2, 4096, 8, 2048
EPS = 1e-6
NCORES = 8
TOK = (B * T) // NCORES          # tokens per core = 1024
SUPER = 128                      # tokens per super-iteration
G = TOK // SUPER                 # super-iterations per core = 8
TPT = 128 // N                   # tokens per tile = 16
J = SUPER // TPT                 # tiles per super-iteration = 8
NT = G * J                       # tiles per core = 64

F32 = mybir.dt.float32
import os
DT16_NAME = os.environ.get("K_DT16", "float16")
FP16 = mybir.dt.float16 if DT16_NAME == "float16" else mybir.dt.bfloat16
NP16 = __import__("numpy").float16 if DT16_NAME == "float16" else __import__("ml_dtypes").bfloat16
FT = mybir.ActivationFunctionType
OP = mybir.AluOpType

# Reduction-pass schedule: which engine does each tile's sumsq / dot.
# 'A' = ScalarE activation(Square), 'V' = VectorE STT, 'P' = GpSimd STT.
SUMSQ_SPLIT = {"A": 60, "V": 4, "P": 0}    # must sum to NT
DOT_SPLIT = {"V": 64, "P": 0}              # must sum to NT


def _spread(split: dict[str, int], n: int) -> list[str]:
    """Interleave engine assignments evenly across n slots."""
    assert sum(split.values()) == n
    acc = {k: 0.0 for k in split}
    out = []
    for _ in range(n):
        for k in acc:
            acc[k] += split[k] / n
        k = max(acc, key=lambda e: acc[e])
        out.append(k)
        acc[k] -= 1.0
    counts = {k: out.count(k) for k in split}
    assert counts == split, (counts, split)
    return out


def _make_schedule(sumsq_split=None, dot_split=None):
    ss = _spread(sumsq_split or SUMSQ_SPLIT, NT)
    dd = _spread(dot_split or DOT_SPLIT, NT)
    return ss, dd


def _split_multi_waits(nc: bass.Bass, limit: int = 1) -> None:
    """Move surplus sync waits onto same-engine NoOp carriers.

    This walrus build accepts only one sync-wait slot per ISA instruction;
    Tile can attach several.  A NoOp on the same engine executed immediately
    before the instruction enforces the same AND-of-waits semantics.
    """
    k = 0
    for func in nc.m.functions:
        for blk in func.blocks:
            new_insts = []
            for inst in blk.instructions:
                si = inst.sync_info
                ow = list(si.on_wait) if si is not None and si.on_wait else []
                if len(ow) > limit:
                    for w in ow[:-limit]:
                        nop = mybir.InstNoOp(
                            name=f"waitnop-{k}",
                            sync_info=mybir.SyncInfo(on_wait=[w], on_update=[]),
                            bass_nofuse=True,
                            engine=inst.engine,
                        )
                        k += 1
                        new_insts.append(nop)
                    si.on_wait = ow[-limit:]
                new_insts.append(inst)
            if len(new_insts) != len(blk.instructions):
                blk.instructions[:] = new_insts


def build_nc(split_waits: bool = True, loop_n: int | None = None,
             store_scalar: bool = True, body_reps: int = 1,
             sumsq_split=None, dot_split=None, xbufs: int = 22,
             spool_bufs: int = 3, wpool_bufs: int = 8,
             opool_bufs: int = 2, mul_pool: bool = True) -> bass.Bass:
    ss_eng, dot_eng = _make_schedule(sumsq_split, dot_split)

    nc = bass.Bass()
    src = nc.declare_dram_parameter("src", [TOK * N, D], FP16, isOutput=False)
    qv = nc.declare_dram_parameter("qv", [D], FP16, isOutput=False)
    maskp = nc.declare_dram_parameter("maskp", [128, J * 128], FP16, isOutput=False)
    onesp = nc.declare_dram_parameter("onesp", [128, 2], FP16, isOutput=False)
    out = nc.declare_dram_parameter("out", [TOK, D], FP16, isOutput=True)

    src_t = src.rearrange("(g j p) d -> g j p d", g=G, j=J, p=128)
    out_t = out.rearrange("(g p) d -> g p d", p=128)

    with tile.TileContext(nc) as tc:
        with (
            tc.tile_pool(name="singles", bufs=1) as singles,
            tc.tile_pool(name="xpool", bufs=xbufs) as xpool,
            tc.tile_pool(name="scr_a", bufs=1) as scr_a,
            tc.tile_pool(name="scr_v", bufs=1) as scr_v,
            tc.tile_pool(name="scr_p", bufs=1) as scr_p,
            tc.tile_pool(name="ypool", bufs=4) as ypool,
            tc.tile_pool(name="spool", bufs=spool_bufs) as spool,
            tc.tile_pool(name="wpool", bufs=wpool_bufs) as wpool,
            tc.tile_pool(name="opool", bufs=opool_bufs) as opool,
            tc.tile_pool(name="psum_o", bufs=1, space="PSUM") as psum_o_pool,
            tc.tile_pool(name="psum_z", bufs=2, space="PSUM") as psum_z_pool,
        ):
            # ---- one-time constants ----
            qb = singles.tile([128, D], FP16)
            nc.sync.dma_start(out=qb, in_=qv[None, :].to_broadcast([128, D]))

            mask = singles.tile([128, J * 128], FP16)
            nc.sync.dma_start(out=mask, in_=maskp[:, :])

            ones_col = singles.tile([128, 2], FP16)
            nc.sync.dma_start(out=ones_col, in_=onesp[:, :])

            bias_eps = singles.tile([128, 1], F32)
            nc.vector.memset(bias_eps, EPS * D)
            bias_zero = singles.tile([128, 1], F32)
            nc.vector.memset(bias_zero, 0.0)

            # Touch qb on VectorE once so later consumers inherit the
            # dependency via engine program order instead of extra sem waits
            # (the TensorScalarPtr ISA slot has a tight wait budget).
            probe = singles.tile([128, 2], F32)
            nc.vector.tensor_copy(probe[:, 0:1], qb[:, 0:1])
            if mul_pool or "P" in ss_eng or "P" in dot_eng or "T" in ss_eng or "T" in dot_eng:
                nc.gpsimd.tensor_copy(probe[:, 1:2], qb[:, 0:1])

            import contextlib

            loop_cm = (
                tc.For_i(0, loop_n, 1,
                         hint_engines=(mybir.EngineType.PE,
                                       mybir.EngineType.Activation,
                                       mybir.EngineType.DVE,
                                       mybir.EngineType.Pool))
                if loop_n is not None
                else contextlib.nullcontext()
            )
            # ---- per-super emission stages (2-deep software pipeline) ----

            def emit_loads_reductions(g):
                sums = spool.tile([128, J], F32, tag="sums")
                dots = spool.tile([128, J], F32, tag="dots")
                xts = []
                for j in range(J):
                    i = g * J + j
                    xt = xpool.tile([128, D], FP16)
                    nc.sync.dma_start(out=xt, in_=src_t[g, j])
                    xts.append(xt)

                    se = ss_eng[i]
                    if se == "A":
                        sq_scr = scr_a.tile([128, D], FP16, tag="sq")
                        nc.scalar.activation(
                            out=sq_scr, in_=xt, func=FT.Square,
                            bias=bias_zero, scale=1.0,
                            accum_out=sums[:, j : j + 1],
                        )
                    elif se == "T":
                        # two-stage: GpSimd squares, DVE tensor_scalar sums
                        # (tensor_scalar+accum packs at fp16; STT does not)
                        y = ypool.tile([128, D], FP16, tag="ysq")
                        nc.gpsimd.tensor_mul(y, xt, xt)
                        scr = scr_v.tile([128, D], FP16, tag="sq")
                        nc.vector.tensor_scalar(
                            out=scr, in0=y, scalar1=1.0, scalar2=1.0,
                            op0=OP.mult, op1=OP.mult,
                            accum_out=sums[:, j : j + 1],
                        )
                    else:
                        eng = nc.vector if se == "V" else nc.gpsimd
                        scr = (scr_v if se == "V" else scr_p).tile(
                            [128, D], FP16, tag="sq")
                        eng.scalar_tensor_tensor(
                            out=scr, in0=xt, scalar=1.0, in1=xt,
                            op0=OP.mult, op1=OP.mult,
                            accum_out=sums[:, j : j + 1],
                        )

                    de = dot_eng[i]
                    if de == "T":
                        y = ypool.tile([128, D], FP16, tag="ydot")
                        nc.gpsimd.tensor_mul(y, xt, qb)
                        scr = scr_v.tile([128, D], FP16, tag="tt")
                        nc.vector.tensor_scalar(
                            out=scr, in0=y, scalar1=1.0, scalar2=1.0,
                            op0=OP.mult, op1=OP.mult,
                            accum_out=dots[:, j : j + 1],
                        )
                    else:
                        eng = nc.vector if de == "V" else nc.gpsimd
                        scr = (scr_v if de == "V" else scr_p).tile(
                            [128, D], FP16, tag="tt")
                        eng.scalar_tensor_tensor(
                            out=scr, in0=xt, scalar=1.0, in1=qb,
                            op0=OP.mult, op1=OP.mult,
                            accum_out=dots[:, j : j + 1],
                        )
                return sums, dots, xts

            def emit_scores(st):
                # score = dot / sqrt(sumsq + eps*D); 1/sqrt = exp(-0.5*ln)
                sums, dots = st["sums"], st["dots"]
                lnv = spool.tile([128, J], F32, tag="lnv")
                nc.scalar.activation(
                    out=lnv, in_=sums, func=FT.Ln, bias=bias_eps, scale=1.0
                )
                rhat = spool.tile([128, J], F32, tag="rhat")
                nc.scalar.activation(
                    out=rhat, in_=lnv, func=FT.Exp, bias=bias_zero, scale=-0.5
                )
                scores = spool.tile([128, J], F32, tag="scores")
                # scores-mul on the otherwise idle GpSimd frees VectorE time
                (nc.gpsimd if mul_pool else nc.vector).tensor_mul(
                    scores, dots, rhat)
                evals = spool.tile([128, J], F32, tag="evals")
                nc.scalar.activation(
                    out=evals, in_=scores, func=FT.Exp, bias=bias_zero
                )
                st["evals"] = evals

            def emit_matmuls(st):
                po = psum_o_pool.tile([128, D], F32)
                pz = psum_z_pool.tile([128, 2], F32)
                evals, xts = st["evals"], st["xts"]
                for j in range(J):
                    w = wpool.tile([128, 128], FP16, tag="w")
                    nc.vector.tensor_scalar_mul(
                        w, mask[:, 128 * j : 128 * (j + 1)],
                        evals[:, j : j + 1],
                    )
                    for c in range(D // 512):
                        nc.tensor.matmul(
                            po[:, 512 * c : 512 * (c + 1)],
                            w,
                            xts[j][:, 512 * c : 512 * (c + 1)],
                            start=(j == 0),
                            stop=(j == J - 1),
                        )
                    nc.tensor.matmul(
                        pz, w, ones_col, start=(j == 0), stop=(j == J - 1)
                    )
                st["po"], st["pz"] = po, pz

            def emit_recip(st):
                invz = spool.tile([128, 1], F32, tag="invz")
                nc.vector.reciprocal(invz, st["pz"][:, 0:1])
                st["invz"] = invz

            def emit_evict(st):
                ot = opool.tile([128, D], FP16)
                nc.scalar.activation(
                    out=ot, in_=st["po"], func=FT.Copy, scale=st["invz"])
                # Store via the scalar-engine HWDGE queue: its wait (evict
                # done) is satisfied by ACT program order, so it never blocks
                # the sync queue's load triggers.
                store_eng = nc.scalar if store_scalar else nc.sync
                store_eng.dma_start(out=out_t[st["g"]], in_=ot)

            with loop_cm:
             # The pipeline carries across body repetitions: the drain (the
             # serialized scores+matmuls+evicts of the last two supers) is
             # paid once per loop body, not once per repetition.
             prev = None   # super g-1: loaded+reduced, needs scores+matmuls
             done = None   # super g-2: matmuls queued, needs recip+evict
             for _rep in range(body_reps):
              for g in range(G):
                sums, dots, xts = emit_loads_reductions(g)
                cur = {"g": g, "sums": sums, "dots": dots, "xts": xts}
                if prev is not None:
                    emit_scores(prev)
                if done is not None:
                    # recip on DVE before ACT needs it for the eviction; the
                    # PSUM source was finished a full super ago, so neither
                    # engine blocks here.
                    emit_recip(done)
                if prev is not None:
                    if done is not None:
                        emit_evict(done)
                    emit_matmuls(prev)
                done, prev = prev, cur
             # drain: scores+matmuls for the last super, evictions for both
             emit_scores(prev)
             emit_recip(done)
             emit_evict(done)
             emit_matmuls(prev)
             emit_recip(prev)
             emit_evict(prev)

    if split_waits:
        _split_multi_waits(nc)
    return nc


def make_mask() -> np.ndarray:
    """Block-diagonal weight scatter masks, one [128, 128] block per tile j.

    Block j has mask[p, TPT*j + p // N] = 1: row p of tile j (= token p//N,
    source p%N) contributes to output token TPT*j + p//N of the super-iter.
    """
    m = np.zeros((128, J * 128), dtype=NP16)
    for j in range(J):
        for p in range(128):
            m[p, 128 * j + TPT * j + p // N] = 1.0
    return m


def kernel(sources, w_query, norm_weight):
    sources = np.asarray(sources, dtype=np.float32)
    w_query = np.asarray(w_query, dtype=np.float32)
    norm_weight = np.asarray(norm_weight, dtype=np.float32)

    nc = build_nc()

    q = np.ascontiguousarray((w_query * norm_weight).astype(NP16))
    flat = np.ascontiguousarray(
        sources.reshape(B * T * N, D).astype(NP16))
    mask_np = make_mask()
    ones_np = np.ones((128, 2), dtype=NP16)
    in_maps = [
        {"src": flat[c * TOK * N : (c + 1) * TOK * N], "qv": q,
         "maskp": mask_np, "onesp": ones_np}
        for c in range(NCORES)
    ]
    global _last_results
    res = run_bass_kernel_spmd(nc, in_maps, list(range(NCORES)), **_run_kwargs)
    _last_results = res
    outs = [res.results[c]["out"] for c in range(NCORES)]
    return (
        np.concatenate(outs, axis=0).reshape(B, T, D).astype(np.float32)
    )


# revision 21
# speedup vs baseline: 1.3091x; 1.0042x over previous
"""Trainium2 Bass kernel for nn_BlockAttentionResidual.

Reference semantics (per (b, t) position):
    inv_rms_n = rsqrt(mean_d(x_n^2) + eps)                 n = 0..7 sources
    score_n   = dot(q, x_n) * inv_rms_n / sqrt(D)          q = w_query * norm_weight
    w         = softmax_n(score_n)
    out       = sum_n w_n * x_n                            [D]

Sharding: 8192 (b,t) tokens split contiguously across 8 cores (1024 each).

The kernel streams fp16 inputs (converted on the host inside kernel(); the
2e-2 tolerance easily covers fp16 rounding, ~5e-4 end-to-end rel err), which
halves the dominant HBM read traffic vs fp32: 32 MiB in + 4 MiB out per core
(~105-120 us of DMA at the ~330 GB/s per-core rate).

Per core, tokens are processed in 8 super-iterations of 128 tokens; each is
J=8 SBUF tiles of [128 rows = 16 tokens x 8 sources, D].  The binding
constraint is the two full-width reduction passes per tile (sum x^2 and
dot(q, x)): on this hardware every reduction-capable op runs at 1 elem/
lane/cycle (DVE scalar_tensor_tensor has no 16-bit packing mode, ScalarE
ACTIVATE is dtype-independent, GpSimd cannot run TensorScalarPtr at all, and
the PE only contracts over partitions so neither reduction can use it), so
the 128 passes are split between ScalarE (activation Square + accum,
~2.36 us) and VectorE (STT + accum, ~2.26 us) by a static schedule tuned on
hardware; the dot can only run on VectorE, which pins DVE at ~150 us and
makes ~165 us the compute floor for this op set.  GpSimd full-width
tensor_tensor measured ~4x slower than its cost-model rate, so it only
carries the tiny per-super scores multiply (dots * rhat, [128, 8]), which
removes a serialization point from the VectorE stream (~8 us).

Emission is software-pipelined two supers deep (reductions for super g,
then scores for g-1, then eviction for g-2, then matmuls for g-1) so the
in-order ACT/DVE instruction streams always have productive work queued
ahead of any cross-engine wait.  Softmax skips max-subtraction:
|score| <= |q| ~ 0.9.  1/sqrt is computed as exp(-0.5*ln(v)) so Square/Ln/
Exp/Copy stay in one ACT table set (no 1.3 us table reloads).  The weighted
combine runs on the PE as PSUM-accumulated matmuls W_j.T @ X_j in fp16
(1 col/cycle, moving operand <= 512 cols for fp16), with W_j a [128, 128]
block-diagonal scatter of exp(score) built by one tensor_scalar_mul against
a constant mask.  The softmax denominator Z accumulates from W_j.T @ ones;
the PSUM->SBUF eviction applies 1/Z via a per-partition activation scale and
emits fp16, stored from the scalar-engine HWDGE queue.
"""

import numpy as np

import concourse.bass as bass
import concourse.tile as tile
from concourse import mybir
from concourse.bass_utils import run_bass_kernel_spmd

# Extra kwargs for run_bass_kernel_spmd (test harness sets {"trace": True});
# the last BassKernelResults is stashed for timing inspection.
_run_kwargs = {}
_last_results = None

B, T, N, D = 2, 4096, 8, 2048
EPS = 1e-6
NCORES = 8
TOK = (B * T) // NCORES          # tokens per core = 1024
SUPER = 128                      # tokens per super-iteration
G = TOK // SUPER                 # super-iterations per core = 8
TPT = 128 // N                   # tokens per tile = 16
J = SUPER // TPT                 # tiles per super-iteration = 8
NT = G * J                       # tiles per core = 64

F32 = mybir.dt.float32
import os
DT16_NAME = os.environ.get("K_DT16", "float16")
FP16 = mybir.dt.float16 if DT16_NAME == "float16" else mybir.dt.bfloat16
NP16 = __import__("numpy").float16 if DT16_NAME == "float16" else __import__("ml_dtypes").bfloat16
FT = mybir.ActivationFunctionType
OP = mybir.AluOpType

# Reduction-pass schedule: which engine does each tile's sumsq / dot.
# 'A' = ScalarE activation(Square), 'V' = VectorE STT, 'P' = GpSimd STT.
SUMSQ_SPLIT = {"A": 60, "V": 4, "P": 0}    # must sum to NT
DOT_SPLIT = {"V": 64, "P": 0}              # must sum to NT


def _spread(split: dict[str, int], n: int) -> list[str]:
    """Interleave engine assignments evenly across n slots."""
    assert sum(split.values()) == n
    acc = {k: 0.0 for k in split}
    out = []
    for _ in range(n):
        for k in acc:
            acc[k] += split[k] / n
        k = max(acc, key=lambda e: acc[e])
        out.append(k)
        acc[k] -= 1.0
    counts = {k: out.count(k) for k in split}
    assert counts == split, (counts, split)
    return out


def _make_schedule(sumsq_split=None, dot_split=None):
    ss = _spread(sumsq_split or SUMSQ_SPLIT, NT)
    dd = _spread(dot_split or DOT_SPLIT, NT)
    return ss, dd


def _split_multi_waits(nc: bass.Bass, limit: int = 1) -> None:
    """Move surplus sync waits onto same-engine NoOp carriers.

    This walrus build accepts only one sync-wait slot per ISA instruction;
    Tile can attach several.  A NoOp on the same engine executed immediately
    before the instruction enforces the same AND-of-waits semantics.
    """
    k = 0
    for func in nc.m.functions:
        for blk in func.blocks:
            new_insts = []
            for inst in blk.instructions:
                si = inst.sync_info
                ow = list(si.on_wait) if si is not None and si.on_wait else []
                if len(ow) > limit:
                    for w in ow[:-limit]:
                        nop = mybir.InstNoOp(
                            name=f"waitnop-{k}",
                            sync_info=mybir.SyncInfo(on_wait=[w], on_update=[]),
                            bass_nofuse=True,
                            engine=inst.engine,
                        )
                        k += 1
                        new_insts.append(nop)
                    si.on_wait = ow[-limit:]
                new_insts.append(inst)
            if len(new_insts) != len(blk.instructions):
                blk.instructions[:] = new_insts


def build_nc(split_waits: bool = True, loop_n: int | None = None,
             store_scalar: bool = True, body_reps: int = 1,
             sumsq_split=None, dot_split=None, xbufs: int = 22,
             spool_bufs: int = 3, wpool_bufs: int = 8,
             opool_bufs: int = 2, mul_pool: bool = True,
             staggered: bool = False) -> bass.Bass:
    ss_eng, dot_eng = _make_schedule(sumsq_split, dot_split)

    nc = bass.Bass()
    src = nc.declare_dram_parameter("src", [TOK * N, D], FP16, isOutput=False)
    qv = nc.declare_dram_parameter("qv", [D], FP16, isOutput=False)
    maskp = nc.declare_dram_parameter("maskp", [128, J * 128], FP16, isOutput=False)
    onesp = nc.declare_dram_parameter("onesp", [128, 2], FP16, isOutput=False)
    out = nc.declare_dram_parameter("out", [TOK, D], FP16, isOutput=True)

    src_t = src.rearrange("(g j p) d -> g j p d", g=G, j=J, p=128)
    out_t = out.rearrange("(g p) d -> g p d", p=128)

    with tile.TileContext(nc) as tc:
        with (
            tc.tile_pool(name="singles", bufs=1) as singles,
            tc.tile_pool(name="xpool", bufs=xbufs) as xpool,
            tc.tile_pool(name="scr_a", bufs=1) as scr_a,
            tc.tile_pool(name="scr_v", bufs=1) as scr_v,
            tc.tile_pool(name="scr_p", bufs=1) as scr_p,
            tc.tile_pool(name="ypool", bufs=4) as ypool,
            tc.tile_pool(name="spool", bufs=spool_bufs) as spool,
            tc.tile_pool(name="wpool", bufs=wpool_bufs) as wpool,
            tc.tile_pool(name="opool", bufs=opool_bufs) as opool,
            tc.tile_pool(name="psum_o", bufs=1, space="PSUM") as psum_o_pool,
            tc.tile_pool(name="psum_z", bufs=2, space="PSUM") as psum_z_pool,
        ):
            # ---- one-time constants ----
            qb = singles.tile([128, D], FP16)
            nc.sync.dma_start(out=qb, in_=qv[None, :].to_broadcast([128, D]))

            mask = singles.tile([128, J * 128], FP16)
            nc.sync.dma_start(out=mask, in_=maskp[:, :])

            ones_col = singles.tile([128, 2], FP16)
            nc.sync.dma_start(out=ones_col, in_=onesp[:, :])

            bias_eps = singles.tile([128, 1], F32)
            nc.vector.memset(bias_eps, EPS * D)
            bias_zero = singles.tile([128, 1], F32)
            nc.vector.memset(bias_zero, 0.0)

            # Touch qb on VectorE once so later consumers inherit the
            # dependency via engine program order instead of extra sem waits
            # (the TensorScalarPtr ISA slot has a tight wait budget).
            probe = singles.tile([128, 2], F32)
            nc.vector.tensor_copy(probe[:, 0:1], qb[:, 0:1])
            if mul_pool or "P" in ss_eng or "P" in dot_eng or "T" in ss_eng or "T" in dot_eng:
                nc.gpsimd.tensor_copy(probe[:, 1:2], qb[:, 0:1])

            import contextlib

            loop_cm = (
                tc.For_i(0, loop_n, 1,
                         staggered_reset=staggered,
                         hint_engines=(mybir.EngineType.PE,
                                       mybir.EngineType.Activation,
                                       mybir.EngineType.DVE,
                                       mybir.EngineType.Pool))
                if loop_n is not None
                else contextlib.nullcontext()
            )
            # ---- per-super emission stages (2-deep software pipeline) ----

            def emit_loads_reductions(g):
                sums = spool.tile([128, J], F32, tag="sums")
                dots = spool.tile([128, J], F32, tag="dots")
                xts = []
                for j in range(J):
                    i = g * J + j
                    xt = xpool.tile([128, D], FP16)
                    nc.sync.dma_start(out=xt, in_=src_t[g, j])
                    xts.append(xt)

                    se = ss_eng[i]
                    if se == "A":
                        sq_scr = scr_a.tile([128, D], FP16, tag="sq")
                        nc.scalar.activation(
                            out=sq_scr, in_=xt, func=FT.Square,
                            bias=bias_zero, scale=1.0,
                            accum_out=sums[:, j : j + 1],
                        )
                    elif se == "T":
                        # two-stage: GpSimd squares, DVE tensor_scalar sums
                        # (tensor_scalar+accum packs at fp16; STT does not)
                        y = ypool.tile([128, D], FP16, tag="ysq")
                        nc.gpsimd.tensor_mul(y, xt, xt)
                        scr = scr_v.tile([128, D], FP16, tag="sq")
                        nc.vector.tensor_scalar(
                            out=scr, in0=y, scalar1=1.0, scalar2=1.0,
                            op0=OP.mult, op1=OP.mult,
                            accum_out=sums[:, j : j + 1],
                        )
                    else:
                        eng = nc.vector if se == "V" else nc.gpsimd
                        scr = (scr_v if se == "V" else scr_p).tile(
                            [128, D], FP16, tag="sq")
                        eng.scalar_tensor_tensor(
                            out=scr, in0=xt, scalar=1.0, in1=xt,
                            op0=OP.mult, op1=OP.mult,
                            accum_out=sums[:, j : j + 1],
                        )

                    de = dot_eng[i]
                    if de == "T":
                        y = ypool.tile([128, D], FP16, tag="ydot")
                        nc.gpsimd.tensor_mul(y, xt, qb)
                        scr = scr_v.tile([128, D], FP16, tag="tt")
                        nc.vector.tensor_scalar(
                            out=scr, in0=y, scalar1=1.0, scalar2=1.0,
                            op0=OP.mult, op1=OP.mult,
                            accum_out=dots[:, j : j + 1],
                        )
                    else:
                        eng = nc.vector if de == "V" else nc.gpsimd
                        scr = (scr_v if de == "V" else scr_p).tile(
                            [128, D], FP16, tag="tt")
                        eng.scalar_tensor_tensor(
                            out=scr, in0=xt, scalar=1.0, in1=qb,
                            op0=OP.mult, op1=OP.mult,
                            accum_out=dots[:, j : j + 1],
                        )
                return sums, dots, xts

            def emit_scores(st):
                # score = dot / sqrt(sumsq + eps*D); 1/sqrt = exp(-0.5*ln)
                sums, dots = st["sums"], st["dots"]
                lnv = spool.tile([128, J], F32, tag="lnv")
                nc.scalar.activation(
                    out=lnv, in_=sums, func=FT.Ln, bias=bias_eps, scale=1.0
                )
                rhat = spool.tile([128, J], F32, tag="rhat")
                nc.scalar.activation(
                    out=rhat, in_=lnv, func=FT.Exp, bias=bias_zero, scale=-0.5
                )
                scores = spool.tile([128, J], F32, tag="scores")
                # scores-mul on the otherwise idle GpSimd frees VectorE time
                (nc.gpsimd if mul_pool else nc.vector).tensor_mul(
                    scores, dots, rhat)
                evals = spool.tile([128, J], F32, tag="evals")
                nc.scalar.activation(
                    out=evals, in_=scores, func=FT.Exp, bias=bias_zero
                )
                st["evals"] = evals

            def emit_matmuls(st):
                po = psum_o_pool.tile([128, D], F32)
                pz = psum_z_pool.tile([128, 2], F32)
                evals, xts = st["evals"], st["xts"]
                for j in range(J):
                    w = wpool.tile([128, 128], FP16, tag="w")
                    nc.vector.tensor_scalar_mul(
                        w, mask[:, 128 * j : 128 * (j + 1)],
                        evals[:, j : j + 1],
                    )
                    for c in range(D // 512):
                        nc.tensor.matmul(
                            po[:, 512 * c : 512 * (c + 1)],
                            w,
                            xts[j][:, 512 * c : 512 * (c + 1)],
                            start=(j == 0),
                            stop=(j == J - 1),
                        )
                    nc.tensor.matmul(
                        pz, w, ones_col, start=(j == 0), stop=(j == J - 1)
                    )
                st["po"], st["pz"] = po, pz

            def emit_recip(st):
                invz = spool.tile([128, 1], F32, tag="invz")
                nc.vector.reciprocal(invz, st["pz"][:, 0:1])
                st["invz"] = invz

            def emit_evict(st):
                ot = opool.tile([128, D], FP16)
                nc.scalar.activation(
                    out=ot, in_=st["po"], func=FT.Copy, scale=st["invz"])
                # Store via the scalar-engine HWDGE queue: its wait (evict
                # done) is satisfied by ACT program order, so it never blocks
                # the sync queue's load triggers.
                store_eng = nc.scalar if store_scalar else nc.sync
                store_eng.dma_start(out=out_t[st["g"]], in_=ot)

            with loop_cm:
             # The pipeline carries across body repetitions: the drain (the
             # serialized scores+matmuls+evicts of the last two supers) is
             # paid once per loop body, not once per repetition.
             prev = None   # super g-1: loaded+reduced, needs scores+matmuls
             done = None   # super g-2: matmuls queued, needs recip+evict
             for _rep in range(body_reps):
              for g in range(G):
                sums, dots, xts = emit_loads_reductions(g)
                cur = {"g": g, "sums": sums, "dots": dots, "xts": xts}
                if prev is not None:
                    emit_scores(prev)
                if done is not None:
                    # recip on DVE before ACT needs it for the eviction; the
                    # PSUM source was finished a full super ago, so neither
                    # engine blocks here.
                    emit_recip(done)
                if prev is not None:
                    if done is not None:
                        emit_evict(done)
                    emit_matmuls(prev)
                done, prev = prev, cur
             # drain: scores+matmuls for the last super, evictions for both
             emit_scores(prev)
             emit_recip(done)
             emit_evict(done)
             emit_matmuls(prev)
             emit_recip(prev)
             emit_evict(prev)

    if split_waits:
        _split_multi_waits(nc)
    return nc


def make_mask() -> np.ndarray:
    """Block-diagonal weight scatter masks, one [128, 128] block per tile j.

    Block j has mask[p, TPT*j + p // N] = 1: row p of tile j (= token p//N,
    source p%N) contributes to output token TPT*j + p//N of the super-iter.
    """
    m = np.zeros((128, J * 128), dtype=NP16)
    for j in range(J):
        for p in range(128):
            m[p, 128 * j + TPT * j + p // N] = 1.0
    return m


def kernel(sources, w_query, norm_weight):
    sources = np.asarray(sources, dtype=np.float32)
    w_query = np.asarray(w_query, dtype=np.float32)
    norm_weight = np.asarray(norm_weight, dtype=np.float32)

    nc = build_nc()

    q = np.ascontiguousarray((w_query * norm_weight).astype(NP16))
    flat = np.ascontiguousarray(
        sources.reshape(B * T * N, D).astype(NP16))
    mask_np = make_mask()
    ones_np = np.ones((128, 2), dtype=NP16)
    in_maps = [
        {"src": flat[c * TOK * N : (c + 1) * TOK * N], "qv": q,
         "maskp": mask_np, "onesp": ones_np}
        for c in range(NCORES)
    ]
    global _last_results
    res = run_bass_kernel_spmd(nc, in_maps, list(range(NCORES)), **_run_kwargs)
    _last_results = res
    outs = [res.results[c]["out"] for c in range(NCORES)]
    return (
        np.concatenate(outs, axis=0).reshape(B, T, D).astype(np.float32)
    )


# revision 22
# speedup vs baseline: 1.3186x; 1.0072x over previous
"""Trainium2 Bass kernel for nn_BlockAttentionResidual.

Reference semantics (per (b, t) position):
    inv_rms_n = rsqrt(mean_d(x_n^2) + eps)                 n = 0..7 sources
    score_n   = dot(q, x_n) * inv_rms_n / sqrt(D)          q = w_query * norm_weight
    w         = softmax_n(score_n)
    out       = sum_n w_n * x_n                            [D]

Sharding: 8192 (b,t) tokens split contiguously across 8 cores (1024 each).

The kernel streams fp16 inputs (converted on the host inside kernel(); the
2e-2 tolerance easily covers fp16 rounding, ~5e-4 end-to-end rel err), which
halves the dominant HBM read traffic vs fp32: 32 MiB in + 4 MiB out per core
(~105-120 us of DMA at the ~330 GB/s per-core rate).

Per core, tokens are processed in 8 super-iterations of 128 tokens; each is
J=8 SBUF tiles of [128 rows = 16 tokens x 8 sources, D].  The binding
constraint is the two full-width reduction passes per tile (sum x^2 and
dot(q, x)): on this hardware every reduction-capable op runs at 1 elem/
lane/cycle (DVE scalar_tensor_tensor has no 16-bit packing mode, ScalarE
ACTIVATE is dtype-independent, GpSimd cannot run TensorScalarPtr at all, and
the PE only contracts over partitions so neither reduction can use it), so
the 128 passes are split between ScalarE (activation Square + accum,
~2.36 us) and VectorE (STT + accum, ~2.26 us) by a static schedule tuned on
hardware; the dot can only run on VectorE, which pins DVE at ~150 us and
makes ~165 us the compute floor for this op set.  GpSimd full-width
tensor_tensor measured ~4x slower than its cost-model rate, so it only
carries the tiny per-super scores multiply (dots * rhat, [128, 8]), which
removes a serialization point from the VectorE stream (~8 us).

Emission is software-pipelined two supers deep (reductions for super g,
then scores for g-1, then eviction for g-2, then matmuls for g-1) so the
in-order ACT/DVE instruction streams always have productive work queued
ahead of any cross-engine wait.  Softmax skips max-subtraction:
|score| <= |q| ~ 0.9.  1/sqrt is computed as exp(-0.5*ln(v)) so Square/Ln/
Exp/Copy stay in one ACT table set (no 1.3 us table reloads).  The weighted
combine runs on the PE as PSUM-accumulated matmuls W_j.T @ X_j in fp16
(1 col/cycle, moving operand <= 512 cols for fp16), with W_j a [128, 128]
block-diagonal scatter of exp(score) built by one tensor_scalar_mul against
a constant mask.  The softmax denominator Z accumulates from W_j.T @ ones;
the PSUM->SBUF eviction applies 1/Z via a per-partition activation scale and
emits fp16, stored from the scalar-engine HWDGE queue.
"""

import numpy as np

import concourse.bass as bass
import concourse.tile as tile
from concourse import mybir
from concourse.bass_utils import run_bass_kernel_spmd

# Extra kwargs for run_bass_kernel_spmd (test harness sets {"trace": True});
# the last BassKernelResults is stashed for timing inspection.
_run_kwargs = {}
_last_results = None

B, T, N, D = 2, 4096, 8, 2048
EPS = 1e-6
NCORES = 8
TOK = (B * T) // NCORES          # tokens per core = 1024
SUPER = 128                      # tokens per super-iteration
G = TOK // SUPER                 # super-iterations per core = 8
TPT = 128 // N                   # tokens per tile = 16
J = SUPER // TPT                 # tiles per super-iteration = 8
NT = G * J                       # tiles per core = 64

F32 = mybir.dt.float32
import os
DT16_NAME = os.environ.get("K_DT16", "float16")
FP16 = mybir.dt.float16 if DT16_NAME == "float16" else mybir.dt.bfloat16
NP16 = __import__("numpy").float16 if DT16_NAME == "float16" else __import__("ml_dtypes").bfloat16
FT = mybir.ActivationFunctionType
OP = mybir.AluOpType

# Reduction-pass schedule: which engine does each tile's sumsq / dot.
# 'A' = ScalarE activation(Square), 'V' = VectorE STT, 'P' = GpSimd STT.
SUMSQ_SPLIT = {"A": 60, "V": 4, "P": 0}    # must sum to NT
DOT_SPLIT = {"V": 64, "P": 0}              # must sum to NT


def _spread(split: dict[str, int], n: int) -> list[str]:
    """Interleave engine assignments evenly across n slots."""
    assert sum(split.values()) == n
    acc = {k: 0.0 for k in split}
    out = []
    for _ in range(n):
        for k in acc:
            acc[k] += split[k] / n
        k = max(acc, key=lambda e: acc[e])
        out.append(k)
        acc[k] -= 1.0
    counts = {k: out.count(k) for k in split}
    assert counts == split, (counts, split)
    return out


def _make_schedule(sumsq_split=None, dot_split=None):
    ss = _spread(sumsq_split or SUMSQ_SPLIT, NT)
    dd = _spread(dot_split or DOT_SPLIT, NT)
    return ss, dd


def _split_multi_waits(nc: bass.Bass, limit: int = 1) -> None:
    """Move surplus sync waits onto same-engine NoOp carriers.

    This walrus build accepts only one sync-wait slot per ISA instruction;
    Tile can attach several.  A NoOp on the same engine executed immediately
    before the instruction enforces the same AND-of-waits semantics.
    """
    k = 0
    for func in nc.m.functions:
        for blk in func.blocks:
            new_insts = []
            for inst in blk.instructions:
                si = inst.sync_info
                ow = list(si.on_wait) if si is not None and si.on_wait else []
                if len(ow) > limit:
                    for w in ow[:-limit]:
                        nop = mybir.InstNoOp(
                            name=f"waitnop-{k}",
                            sync_info=mybir.SyncInfo(on_wait=[w], on_update=[]),
                            bass_nofuse=True,
                            engine=inst.engine,
                        )
                        k += 1
                        new_insts.append(nop)
                    si.on_wait = ow[-limit:]
                new_insts.append(inst)
            if len(new_insts) != len(blk.instructions):
                blk.instructions[:] = new_insts


def build_nc(split_waits: bool = True, loop_n: int | None = None,
             store_scalar: bool = True, body_reps: int = 1,
             sumsq_split=None, dot_split=None, xbufs: int = 22,
             spool_bufs: int = 3, wpool_bufs: int = 8,
             opool_bufs: int = 2, mul_pool: bool = True,
             staggered: bool = False) -> bass.Bass:
    ss_eng, dot_eng = _make_schedule(sumsq_split, dot_split)

    nc = bass.Bass()
    src = nc.declare_dram_parameter("src", [TOK * N, D], FP16, isOutput=False)
    qv = nc.declare_dram_parameter("qv", [D], FP16, isOutput=False)
    maskp = nc.declare_dram_parameter("maskp", [128, J * 128], FP16, isOutput=False)
    onesp = nc.declare_dram_parameter("onesp", [128, 2], FP16, isOutput=False)
    out = nc.declare_dram_parameter("out", [TOK, D], FP16, isOutput=True)

    src_t = src.rearrange("(g j p) d -> g j p d", g=G, j=J, p=128)
    out_t = out.rearrange("(g p) d -> g p d", p=128)

    with tile.TileContext(nc) as tc:
        with (
            tc.tile_pool(name="singles", bufs=1) as singles,
            tc.tile_pool(name="xpool", bufs=xbufs) as xpool,
            tc.tile_pool(name="scr_a", bufs=1) as scr_a,
            tc.tile_pool(name="scr_v", bufs=1) as scr_v,
            tc.tile_pool(name="scr_p", bufs=1) as scr_p,
            tc.tile_pool(name="ypool", bufs=4) as ypool,
            tc.tile_pool(name="spool", bufs=spool_bufs) as spool,
            tc.tile_pool(name="wpool", bufs=wpool_bufs) as wpool,
            tc.tile_pool(name="opool", bufs=opool_bufs) as opool,
            tc.tile_pool(name="psum_o", bufs=1, space="PSUM") as psum_o_pool,
            tc.tile_pool(name="psum_z", bufs=2, space="PSUM") as psum_z_pool,
        ):
            # ---- one-time constants ----
            qb = singles.tile([128, D], FP16)
            nc.sync.dma_start(out=qb, in_=qv[None, :].to_broadcast([128, D]))

            mask = singles.tile([128, J * 128], FP16)
            nc.sync.dma_start(out=mask, in_=maskp[:, :])

            ones_col = singles.tile([128, 2], FP16)
            nc.sync.dma_start(out=ones_col, in_=onesp[:, :])

            bias_eps = singles.tile([128, 1], F32)
            nc.vector.memset(bias_eps, EPS * D)
            bias_zero = singles.tile([128, 1], F32)
            nc.vector.memset(bias_zero, 0.0)

            # Touch qb on VectorE once so later consumers inherit the
            # dependency via engine program order instead of extra sem waits
            # (the TensorScalarPtr ISA slot has a tight wait budget).
            probe = singles.tile([128, 2], F32)
            nc.vector.tensor_copy(probe[:, 0:1], qb[:, 0:1])
            if mul_pool or "P" in ss_eng or "P" in dot_eng or "T" in ss_eng or "T" in dot_eng:
                nc.gpsimd.tensor_copy(probe[:, 1:2], qb[:, 0:1])

            import contextlib

            loop_cm = (
                tc.For_i(0, loop_n, 1,
                         staggered_reset=staggered,
                         hint_engines=(mybir.EngineType.PE,
                                       mybir.EngineType.Activation,
                                       mybir.EngineType.DVE,
                                       mybir.EngineType.Pool))
                if loop_n is not None
                else contextlib.nullcontext()
            )
            # ---- per-super emission stages (2-deep software pipeline) ----

            def emit_loads_reductions(g):
                sums = spool.tile([128, J], F32, tag="sums")
                dots = spool.tile([128, J], F32, tag="dots")
                xts = []
                for j in range(J):
                    i = g * J + j
                    xt = xpool.tile([128, D], FP16)
                    nc.sync.dma_start(out=xt, in_=src_t[g, j])
                    xts.append(xt)

                    se = ss_eng[i]
                    if se == "A":
                        sq_scr = scr_a.tile([128, D], FP16, tag="sq")
                        nc.scalar.activation(
                            out=sq_scr, in_=xt, func=FT.Square,
                            bias=bias_zero, scale=1.0,
                            accum_out=sums[:, j : j + 1],
                        )
                    elif se == "U":
                        y = ypool.tile([128, D], FP16, tag="ysq")
                        nc.vector.tensor_mul(y, xt, xt)
                        scr = scr_v.tile([128, D], FP16, tag="sq")
                        nc.vector.tensor_scalar(
                            out=scr, in0=y, scalar1=1.0, scalar2=1.0,
                            op0=OP.mult, op1=OP.mult,
                            accum_out=sums[:, j : j + 1],
                        )
                    elif se == "T":
                        # two-stage: GpSimd squares, DVE tensor_scalar sums
                        # (tensor_scalar+accum packs at fp16; STT does not)
                        y = ypool.tile([128, D], FP16, tag="ysq")
                        nc.gpsimd.tensor_mul(y, xt, xt)
                        scr = scr_v.tile([128, D], FP16, tag="sq")
                        nc.vector.tensor_scalar(
                            out=scr, in0=y, scalar1=1.0, scalar2=1.0,
                            op0=OP.mult, op1=OP.mult,
                            accum_out=sums[:, j : j + 1],
                        )
                    else:
                        eng = nc.vector if se == "V" else nc.gpsimd
                        scr = (scr_v if se == "V" else scr_p).tile(
                            [128, D], FP16, tag="sq")
                        eng.scalar_tensor_tensor(
                            out=scr, in0=xt, scalar=1.0, in1=xt,
                            op0=OP.mult, op1=OP.mult,
                            accum_out=sums[:, j : j + 1],
                        )

                    de = dot_eng[i]
                    if de == "U":
                        # 2-op DVE recipe: TT mult at 2x, then 1-src
                        # tensor_scalar+accum (4x if packing holds)
                        y = ypool.tile([128, D], FP16, tag="ydot")
                        nc.vector.tensor_mul(y, xt, qb)
                        scr = scr_v.tile([128, D], FP16, tag="tt")
                        nc.vector.tensor_scalar(
                            out=scr, in0=y, scalar1=1.0, scalar2=1.0,
                            op0=OP.mult, op1=OP.mult,
                            accum_out=dots[:, j : j + 1],
                        )
                    elif de == "T":
                        y = ypool.tile([128, D], FP16, tag="ydot")
                        nc.gpsimd.tensor_mul(y, xt, qb)
                        scr = scr_v.tile([128, D], FP16, tag="tt")
                        nc.vector.tensor_scalar(
                            out=scr, in0=y, scalar1=1.0, scalar2=1.0,
                            op0=OP.mult, op1=OP.mult,
                            accum_out=dots[:, j : j + 1],
                        )
                    else:
                        eng = nc.vector if de == "V" else nc.gpsimd
                        scr = (scr_v if de == "V" else scr_p).tile(
                            [128, D], FP16, tag="tt")
                        eng.scalar_tensor_tensor(
                            out=scr, in0=xt, scalar=1.0, in1=qb,
                            op0=OP.mult, op1=OP.mult,
                            accum_out=dots[:, j : j + 1],
                        )
                return sums, dots, xts

            def emit_scores(st):
                # score = dot / sqrt(sumsq + eps*D); 1/sqrt = exp(-0.5*ln)
                sums, dots = st["sums"], st["dots"]
                lnv = spool.tile([128, J], F32, tag="lnv")
                nc.scalar.activation(
                    out=lnv, in_=sums, func=FT.Ln, bias=bias_eps, scale=1.0
                )
                rhat = spool.tile([128, J], F32, tag="rhat")
                nc.scalar.activation(
                    out=rhat, in_=lnv, func=FT.Exp, bias=bias_zero, scale=-0.5
                )
                scores = spool.tile([128, J], F32, tag="scores")
                # scores-mul on the otherwise idle GpSimd frees VectorE time
                (nc.gpsimd if mul_pool else nc.vector).tensor_mul(
                    scores, dots, rhat)
                evals = spool.tile([128, J], F32, tag="evals")
                nc.scalar.activation(
                    out=evals, in_=scores, func=FT.Exp, bias=bias_zero
                )
                st["evals"] = evals

            def emit_matmuls(st):
                po = psum_o_pool.tile([128, D], F32)
                pz = psum_z_pool.tile([128, 2], F32)
                evals, xts = st["evals"], st["xts"]
                for j in range(J):
                    w = wpool.tile([128, 128], FP16, tag="w")
                    nc.vector.tensor_scalar_mul(
                        w, mask[:, 128 * j : 128 * (j + 1)],
                        evals[:, j : j + 1],
                    )
                    for c in range(D // 512):
                        nc.tensor.matmul(
                            po[:, 512 * c : 512 * (c + 1)],
                            w,
                            xts[j][:, 512 * c : 512 * (c + 1)],
                            start=(j == 0),
                            stop=(j == J - 1),
                        )
                    nc.tensor.matmul(
                        pz, w, ones_col, start=(j == 0), stop=(j == J - 1)
                    )
                st["po"], st["pz"] = po, pz

            def emit_recip(st):
                invz = spool.tile([128, 1], F32, tag="invz")
                nc.vector.reciprocal(invz, st["pz"][:, 0:1])
                st["invz"] = invz

            def emit_evict(st):
                ot = opool.tile([128, D], FP16)
                nc.scalar.activation(
                    out=ot, in_=st["po"], func=FT.Copy, scale=st["invz"])
                # Store via the scalar-engine HWDGE queue: its wait (evict
                # done) is satisfied by ACT program order, so it never blocks
                # the sync queue's load triggers.
                store_eng = nc.scalar if store_scalar else nc.sync
                store_eng.dma_start(out=out_t[st["g"]], in_=ot)

            with loop_cm:
             # The pipeline carries across body repetitions: the drain (the
             # serialized scores+matmuls+evicts of the last two supers) is
             # paid once per loop body, not once per repetition.
             prev = None   # super g-1: loaded+reduced, needs scores+matmuls
             done = None   # super g-2: matmuls queued, needs recip+evict
             for _rep in range(body_reps):
              for g in range(G):
                sums, dots, xts = emit_loads_reductions(g)
                cur = {"g": g, "sums": sums, "dots": dots, "xts": xts}
                if prev is not None:
                    emit_scores(prev)
                if done is not None:
                    # recip on DVE before ACT needs it for the eviction; the
                    # PSUM source was finished a full super ago, so neither
                    # engine blocks here.
                    emit_recip(done)
                if prev is not None:
                    if done is not None:
                        emit_evict(done)
                    emit_matmuls(prev)
                done, prev = prev, cur
             # drain: scores+matmuls for the last super, evictions for both
             emit_scores(prev)
             emit_recip(done)
             emit_evict(done)
             emit_matmuls(prev)
             emit_recip(prev)
             emit_evict(prev)

    if split_waits:
        _split_multi_waits(nc)
    return nc


def make_mask() -> np.ndarray:
    """Block-diagonal weight scatter masks, one [128, 128] block per tile j.

    Block j has mask[p, TPT*j + p // N] = 1: row p of tile j (= token p//N,
    source p%N) contributes to output token TPT*j + p//N of the super-iter.
    """
    m = np.zeros((128, J * 128), dtype=NP16)
    for j in range(J):
        for p in range(128):
            m[p, 128 * j + TPT * j + p // N] = 1.0
    return m


def kernel(sources, w_query, norm_weight):
    sources = np.asarray(sources, dtype=np.float32)
    w_query = np.asarray(w_query, dtype=np.float32)
    norm_weight = np.asarray(norm_weight, dtype=np.float32)

    nc = build_nc()

    q = np.ascontiguousarray((w_query * norm_weight).astype(NP16))
    flat = np.ascontiguousarray(
        sources.reshape(B * T * N, D).astype(NP16))
    mask_np = make_mask()
    ones_np = np.ones((128, 2), dtype=NP16)
    in_maps = [
        {"src": flat[c * TOK * N : (c + 1) * TOK * N], "qv": q,
         "maskp": mask_np, "onesp": ones_np}
        for c in range(NCORES)
    ]
    global _last_results
    res = run_bass_kernel_spmd(nc, in_maps, list(range(NCORES)), **_run_kwargs)
    _last_results = res
    outs = [res.results[c]["out"] for c in range(NCORES)]
    return (
        np.concatenate(outs, axis=0).reshape(B, T, D).astype(np.float32)
    )


# revision 26
# speedup vs baseline: 1.3264x; 1.0059x over previous
"""Trainium2 Bass kernel for nn_BlockAttentionResidual.

Reference semantics (per (b, t) position):
    inv_rms_n = rsqrt(mean_d(x_n^2) + eps)                 n = 0..7 sources
    score_n   = dot(q, x_n) * inv_rms_n / sqrt(D)          q = w_query * norm_weight
    w         = softmax_n(score_n)
    out       = sum_n w_n * x_n                            [D]

Sharding: 8192 (b,t) tokens split contiguously across 8 cores (1024 each).

The kernel streams fp16 inputs (converted on the host inside kernel(); the
2e-2 tolerance easily covers fp16 rounding, ~5e-4 end-to-end rel err), which
halves the dominant HBM read traffic vs fp32: 32 MiB in + 4 MiB out per core
(~105-120 us of DMA at the ~330 GB/s per-core rate).

Per core, tokens are processed in 8 super-iterations of 128 tokens; each is
J=8 SBUF tiles of [128 rows = 16 tokens x 8 sources, D].  The binding
constraint is the two full-width reduction passes per tile (sum x^2 and
dot(q, x)): on this hardware every reduction-capable op runs at 1 elem/
lane/cycle (DVE scalar_tensor_tensor has no 16-bit packing mode, ScalarE
ACTIVATE is dtype-independent, GpSimd cannot run TensorScalarPtr at all, and
the PE only contracts over partitions so neither reduction can use it), so
the 128 passes are split between ScalarE (activation Square + accum,
~2.36 us) and VectorE (STT + accum, ~2.26 us) by a static schedule tuned on
hardware; the dot can only run on VectorE, which pins DVE at ~150 us and
makes ~165 us the compute floor for this op set.  GpSimd full-width
tensor_tensor measured ~4x slower than its cost-model rate, so it only
carries the tiny per-super scores multiply (dots * rhat, [128, 8]), which
removes a serialization point from the VectorE stream (~8 us).

Emission is software-pipelined two supers deep (reductions for super g,
then scores for g-1, then eviction for g-2, then matmuls for g-1) so the
in-order ACT/DVE instruction streams always have productive work queued
ahead of any cross-engine wait.  Softmax skips max-subtraction:
|score| <= |q| ~ 0.9.  1/sqrt is computed as exp(-0.5*ln(v)) so Square/Ln/
Exp/Copy stay in one ACT table set (no 1.3 us table reloads).  The weighted
combine runs on the PE as PSUM-accumulated matmuls W_j.T @ X_j in fp16
(1 col/cycle, moving operand <= 512 cols for fp16), with W_j a [128, 128]
block-diagonal scatter of exp(score) built by one tensor_scalar_mul against
a constant mask.  The softmax denominator Z accumulates from W_j.T @ ones;
the PSUM->SBUF eviction applies 1/Z via a per-partition activation scale and
emits fp16, stored from the scalar-engine HWDGE queue.
"""

import numpy as np

import concourse.bass as bass
import concourse.tile as tile
from concourse import mybir
from concourse.bass_utils import run_bass_kernel_spmd

# Extra kwargs for run_bass_kernel_spmd (test harness sets {"trace": True});
# the last BassKernelResults is stashed for timing inspection.
_run_kwargs = {}
_last_results = None

B, T, N, D = 2, 4096, 8, 2048
EPS = 1e-6
NCORES = 8
TOK = (B * T) // NCORES          # tokens per core = 1024
SUPER = 128                      # tokens per super-iteration
G = TOK // SUPER                 # super-iterations per core = 8
TPT = 128 // N                   # tokens per tile = 16
J = SUPER // TPT                 # tiles per super-iteration = 8
NT = G * J                       # tiles per core = 64

F32 = mybir.dt.float32
import os
DT16_NAME = os.environ.get("K_DT16", "float16")
FP16 = mybir.dt.float16 if DT16_NAME == "float16" else mybir.dt.bfloat16
NP16 = __import__("numpy").float16 if DT16_NAME == "float16" else __import__("ml_dtypes").bfloat16
FT = mybir.ActivationFunctionType
OP = mybir.AluOpType

# Reduction-pass schedule: which engine does each tile's sumsq / dot.
# 'A' = ScalarE activation(Square), 'V' = VectorE STT, 'P' = GpSimd STT.
SUMSQ_SPLIT = {"A": 60, "V": 4, "P": 0}    # must sum to NT
DOT_SPLIT = {"V": 64, "P": 0}              # must sum to NT


def _spread(split: dict[str, int], n: int) -> list[str]:
    """Interleave engine assignments evenly across n slots."""
    assert sum(split.values()) == n
    acc = {k: 0.0 for k in split}
    out = []
    for _ in range(n):
        for k in acc:
            acc[k] += split[k] / n
        k = max(acc, key=lambda e: acc[e])
        out.append(k)
        acc[k] -= 1.0
    counts = {k: out.count(k) for k in split}
    assert counts == split, (counts, split)
    return out


def _make_schedule(sumsq_split=None, dot_split=None):
    ss = _spread(sumsq_split or SUMSQ_SPLIT, NT)
    dd = _spread(dot_split or DOT_SPLIT, NT)
    return ss, dd


def _split_multi_waits(nc: bass.Bass, limit: int = 1) -> None:
    """Move surplus sync waits onto same-engine NoOp carriers.

    This walrus build accepts only one sync-wait slot per ISA instruction;
    Tile can attach several.  A NoOp on the same engine executed immediately
    before the instruction enforces the same AND-of-waits semantics.
    """
    k = 0
    for func in nc.m.functions:
        for blk in func.blocks:
            new_insts = []
            for inst in blk.instructions:
                si = inst.sync_info
                ow = list(si.on_wait) if si is not None and si.on_wait else []
                if len(ow) > limit:
                    for w in ow[:-limit]:
                        nop = mybir.InstNoOp(
                            name=f"waitnop-{k}",
                            sync_info=mybir.SyncInfo(on_wait=[w], on_update=[]),
                            bass_nofuse=True,
                            engine=inst.engine,
                        )
                        k += 1
                        new_insts.append(nop)
                    si.on_wait = ow[-limit:]
                new_insts.append(inst)
            if len(new_insts) != len(blk.instructions):
                blk.instructions[:] = new_insts


def build_nc(split_waits: bool = True, loop_n: int | None = None,
             store_scalar: bool = True, body_reps: int = 1,
             sumsq_split=None, dot_split=None, xbufs: int = 22,
             spool_bufs: int = 3, wpool_bufs: int = 8,
             opool_bufs: int = 2, mul_pool: bool = True,
             staggered: bool = False, sumsq_half: bool = False,
             split_col: int = 960) -> bass.Bass:
    ss_eng, dot_eng = _make_schedule(sumsq_split, dot_split)

    nc = bass.Bass()
    src = nc.declare_dram_parameter("src", [TOK * N, D], FP16, isOutput=False)
    qv = nc.declare_dram_parameter("qv", [D], FP16, isOutput=False)
    maskp = nc.declare_dram_parameter("maskp", [128, J * 128], FP16, isOutput=False)
    onesp = nc.declare_dram_parameter("onesp", [128, 2], FP16, isOutput=False)
    out = nc.declare_dram_parameter("out", [TOK, D], FP16, isOutput=True)

    src_t = src.rearrange("(g j p) d -> g j p d", g=G, j=J, p=128)
    out_t = out.rearrange("(g p) d -> g p d", p=128)

    with tile.TileContext(nc) as tc:
        with (
            tc.tile_pool(name="singles", bufs=1) as singles,
            tc.tile_pool(name="xpool", bufs=xbufs) as xpool,
            tc.tile_pool(name="scr_a", bufs=1) as scr_a,
            tc.tile_pool(name="scr_v", bufs=1) as scr_v,
            tc.tile_pool(name="scr_p", bufs=1) as scr_p,
            tc.tile_pool(name="ypool", bufs=4) as ypool,
            tc.tile_pool(name="spool", bufs=spool_bufs) as spool,
            tc.tile_pool(name="wpool", bufs=wpool_bufs) as wpool,
            tc.tile_pool(name="opool", bufs=opool_bufs) as opool,
            tc.tile_pool(name="psum_o", bufs=1, space="PSUM") as psum_o_pool,
            tc.tile_pool(name="psum_z", bufs=2, space="PSUM") as psum_z_pool,
        ):
            # ---- one-time constants ----
            qb = singles.tile([128, D], FP16)
            nc.sync.dma_start(out=qb, in_=qv[None, :].to_broadcast([128, D]))

            mask = singles.tile([128, J * 128], FP16)
            nc.sync.dma_start(out=mask, in_=maskp[:, :])

            ones_col = singles.tile([128, 2], FP16)
            nc.sync.dma_start(out=ones_col, in_=onesp[:, :])

            bias_eps = singles.tile([128, 1], F32)
            nc.vector.memset(bias_eps, EPS * D)
            bias_zero = singles.tile([128, 1], F32)
            nc.vector.memset(bias_zero, 0.0)

            # Touch qb on VectorE once so later consumers inherit the
            # dependency via engine program order instead of extra sem waits
            # (the TensorScalarPtr ISA slot has a tight wait budget).
            probe = singles.tile([128, 2], F32)
            nc.vector.tensor_copy(probe[:, 0:1], qb[:, 0:1])
            if mul_pool or "P" in ss_eng or "P" in dot_eng or "T" in ss_eng or "T" in dot_eng:
                nc.gpsimd.tensor_copy(probe[:, 1:2], qb[:, 0:1])

            import contextlib

            loop_cm = (
                tc.For_i(0, loop_n, 1,
                         staggered_reset=staggered,
                         hint_engines=(mybir.EngineType.PE,
                                       mybir.EngineType.Activation,
                                       mybir.EngineType.DVE,
                                       mybir.EngineType.Pool))
                if loop_n is not None
                else contextlib.nullcontext()
            )
            # ---- per-super emission stages (2-deep software pipeline) ----

            def emit_loads_reductions(g):
                sums = spool.tile([128, J], F32, tag="sums")
                dots = spool.tile([128, J], F32, tag="dots")
                sums2 = None
                if sumsq_half:
                    sums2 = spool.tile([128, 1], F32, tag="sums2")
                xts = []
                for j in range(J):
                    i = g * J + j
                    xt = xpool.tile([128, D], FP16)
                    nc.sync.dma_start(out=xt, in_=src_t[g, j])
                    xts.append(xt)

                    # Balanced-bundle mode: every super gets 7 full ACT
                    # squares; the last tile's sumsq is split at split_col
                    # between ACT and DVE so both engines carry an identical
                    # per-super load (no integer jitter at the per-super
                    # scores barrier).
                    se = ss_eng[i]
                    if sumsq_half:
                        se = "A" if j < J - 1 else "H"
                    if se == "H":
                        sq_scr = scr_a.tile([128, D], FP16, tag="sq")
                        nc.scalar.activation(
                            out=sq_scr[:, :split_col],
                            in_=xt[:, :split_col], func=FT.Square,
                            bias=bias_zero, scale=1.0,
                            accum_out=sums[:, j : j + 1],
                        )
                        de = dot_eng[i]
                        eng = nc.vector if de == "V" else nc.gpsimd
                        scr = (scr_v if de == "V" else scr_p).tile(
                            [128, D], FP16, tag="tt")
                        eng.scalar_tensor_tensor(
                            out=scr, in0=xt, scalar=1.0, in1=qb,
                            op0=OP.mult, op1=OP.mult,
                            accum_out=dots[:, j : j + 1],
                        )
                        scr2 = scr_v.tile([128, D], FP16, tag="sqh")
                        nc.vector.scalar_tensor_tensor(
                            out=scr2[:, split_col:], in0=xt[:, split_col:],
                            scalar=1.0, in1=xt[:, split_col:],
                            op0=OP.mult, op1=OP.mult,
                            accum_out=sums2[:, 0:1],
                        )
                        continue
                    if se == "A":
                        sq_scr = scr_a.tile([128, D], FP16, tag="sq")
                        nc.scalar.activation(
                            out=sq_scr, in_=xt, func=FT.Square,
                            bias=bias_zero, scale=1.0,
                            accum_out=sums[:, j : j + 1],
                        )
                    elif se == "U":
                        y = ypool.tile([128, D], FP16, tag="ysq")
                        nc.vector.tensor_mul(y, xt, xt)
                        scr = scr_v.tile([128, D], FP16, tag="sq")
                        nc.vector.tensor_scalar(
                            out=scr, in0=y, scalar1=1.0, scalar2=1.0,
                            op0=OP.mult, op1=OP.mult,
                            accum_out=sums[:, j : j + 1],
                        )
                    elif se == "T":
                        # two-stage: GpSimd squares, DVE tensor_scalar sums
                        # (tensor_scalar+accum packs at fp16; STT does not)
                        y = ypool.tile([128, D], FP16, tag="ysq")
                        nc.gpsimd.tensor_mul(y, xt, xt)
                        scr = scr_v.tile([128, D], FP16, tag="sq")
                        nc.vector.tensor_scalar(
                            out=scr, in0=y, scalar1=1.0, scalar2=1.0,
                            op0=OP.mult, op1=OP.mult,
                            accum_out=sums[:, j : j + 1],
                        )
                    else:
                        eng = nc.vector if se == "V" else nc.gpsimd
                        scr = (scr_v if se == "V" else scr_p).tile(
                            [128, D], FP16, tag="sq")
                        eng.scalar_tensor_tensor(
                            out=scr, in0=xt, scalar=1.0, in1=xt,
                            op0=OP.mult, op1=OP.mult,
                            accum_out=sums[:, j : j + 1],
                        )

                    de = dot_eng[i]
                    if de == "U":
                        # 2-op DVE recipe: TT mult at 2x, then 1-src
                        # tensor_scalar+accum (4x if packing holds)
                        y = ypool.tile([128, D], FP16, tag="ydot")
                        nc.vector.tensor_mul(y, xt, qb)
                        scr = scr_v.tile([128, D], FP16, tag="tt")
                        nc.vector.tensor_scalar(
                            out=scr, in0=y, scalar1=1.0, scalar2=1.0,
                            op0=OP.mult, op1=OP.mult,
                            accum_out=dots[:, j : j + 1],
                        )
                    elif de == "T":
                        y = ypool.tile([128, D], FP16, tag="ydot")
                        nc.gpsimd.tensor_mul(y, xt, qb)
                        scr = scr_v.tile([128, D], FP16, tag="tt")
                        nc.vector.tensor_scalar(
                            out=scr, in0=y, scalar1=1.0, scalar2=1.0,
                            op0=OP.mult, op1=OP.mult,
                            accum_out=dots[:, j : j + 1],
                        )
                    else:
                        eng = nc.vector if de == "V" else nc.gpsimd
                        scr = (scr_v if de == "V" else scr_p).tile(
                            [128, D], FP16, tag="tt")
                        eng.scalar_tensor_tensor(
                            out=scr, in0=xt, scalar=1.0, in1=qb,
                            op0=OP.mult, op1=OP.mult,
                            accum_out=dots[:, j : j + 1],
                        )
                return sums, dots, sums2, xts

            def emit_scores(st):
                # score = dot / sqrt(sumsq + eps*D); 1/sqrt = exp(-0.5*ln)
                sums, dots = st["sums"], st["dots"]
                if st.get("sums2") is not None:
                    # merge the split tile's two partial accumulators
                    nc.gpsimd.tensor_add(
                        sums[:, J - 1 : J], sums[:, J - 1 : J],
                        st["sums2"])
                lnv = spool.tile([128, J], F32, tag="lnv")
                nc.scalar.activation(
                    out=lnv, in_=sums, func=FT.Ln, bias=bias_eps, scale=1.0
                )
                rhat = spool.tile([128, J], F32, tag="rhat")
                nc.scalar.activation(
                    out=rhat, in_=lnv, func=FT.Exp, bias=bias_zero, scale=-0.5
                )
                scores = spool.tile([128, J], F32, tag="scores")
                # scores-mul on the otherwise idle GpSimd frees VectorE time
                (nc.gpsimd if mul_pool else nc.vector).tensor_mul(
                    scores, dots, rhat)
                evals = spool.tile([128, J], F32, tag="evals")
                nc.scalar.activation(
                    out=evals, in_=scores, func=FT.Exp, bias=bias_zero
                )
                st["evals"] = evals

            def emit_matmuls(st):
                po = psum_o_pool.tile([128, D], F32)
                pz = psum_z_pool.tile([128, 2], F32)
                evals, xts = st["evals"], st["xts"]
                for j in range(J):
                    w = wpool.tile([128, 128], FP16, tag="w")
                    nc.vector.tensor_scalar_mul(
                        w, mask[:, 128 * j : 128 * (j + 1)],
                        evals[:, j : j + 1],
                    )
                    for c in range(D // 512):
                        nc.tensor.matmul(
                            po[:, 512 * c : 512 * (c + 1)],
                            w,
                            xts[j][:, 512 * c : 512 * (c + 1)],
                            start=(j == 0),
                            stop=(j == J - 1),
                        )
                    nc.tensor.matmul(
                        pz, w, ones_col, start=(j == 0), stop=(j == J - 1)
                    )
                st["po"], st["pz"] = po, pz

            def emit_recip(st):
                invz = spool.tile([128, 1], F32, tag="invz")
                nc.vector.reciprocal(invz, st["pz"][:, 0:1])
                st["invz"] = invz

            def emit_evict(st):
                ot = opool.tile([128, D], FP16)
                nc.scalar.activation(
                    out=ot, in_=st["po"], func=FT.Copy, scale=st["invz"])
                # Store via the scalar-engine HWDGE queue: its wait (evict
                # done) is satisfied by ACT program order, so it never blocks
                # the sync queue's load triggers.
                store_eng = nc.scalar if store_scalar else nc.sync
                store_eng.dma_start(out=out_t[st["g"]], in_=ot)

            with loop_cm:
             # The pipeline carries across body repetitions: the drain (the
             # serialized scores+matmuls+evicts of the last two supers) is
             # paid once per loop body, not once per repetition.
             prev = None   # super g-1: loaded+reduced, needs scores+matmuls
             done = None   # super g-2: matmuls queued, needs recip+evict
             for _rep in range(body_reps):
              for g in range(G):
                sums, dots, sums2, xts = emit_loads_reductions(g)
                cur = {"g": g, "sums": sums, "dots": dots, "sums2": sums2,
                       "xts": xts}
                if prev is not None:
                    emit_scores(prev)
                if done is not None:
                    # recip on DVE before ACT needs it for the eviction; the
                    # PSUM source was finished a full super ago, so neither
                    # engine blocks here.
                    emit_recip(done)
                if prev is not None:
                    if done is not None:
                        emit_evict(done)
                    emit_matmuls(prev)
                done, prev = prev, cur
             # drain: scores+matmuls for the last super, evictions for both
             emit_scores(prev)
             emit_recip(done)
             emit_evict(done)
             emit_matmuls(prev)
             emit_recip(prev)
             emit_evict(prev)

    if split_waits:
        _split_multi_waits(nc)
    return nc


def make_mask() -> np.ndarray:
    """Block-diagonal weight scatter masks, one [128, 128] block per tile j.

    Block j has mask[p, TPT*j + p // N] = 1: row p of tile j (= token p//N,
    source p%N) contributes to output token TPT*j + p//N of the super-iter.
    """
    m = np.zeros((128, J * 128), dtype=NP16)
    for j in range(J):
        for p in range(128):
            m[p, 128 * j + TPT * j + p // N] = 1.0
    return m


def kernel(sources, w_query, norm_weight):
    sources = np.asarray(sources, dtype=np.float32)
    w_query = np.asarray(w_query, dtype=np.float32)
    norm_weight = np.asarray(norm_weight, dtype=np.float32)

    nc = build_nc()

    q = np.ascontiguousarray((w_query * norm_weight).astype(NP16))
    flat = np.ascontiguousarray(
        sources.reshape(B * T * N, D).astype(NP16))
    mask_np = make_mask()
    ones_np = np.ones((128, 2), dtype=NP16)
    in_maps = [
        {"src": flat[c * TOK * N : (c + 1) * TOK * N], "qv": q,
         "maskp": mask_np, "onesp": ones_np}
        for c in range(NCORES)
    ]
    global _last_results
    res = run_bass_kernel_spmd(nc, in_maps, list(range(NCORES)), **_run_kwargs)
    _last_results = res
    outs = [res.results[c]["out"] for c in range(NCORES)]
    return (
        np.concatenate(outs, axis=0).reshape(B, T, D).astype(np.float32)
    )
